# revision 70
# baseline (speedup 1.0000x reference)
"""Trainium2 Bass kernel for BHS_GCN: 2x GCNConv + dueling value/advantage heads.

Strategy (8 NeuronCores, single NEFF launch, bf16 compute / fp32 PSUM):
  - GCN phase batch-parallel: each core owns B_LOC=4 full graphs.
    Message passing = per-tile dma_gather of source-node rows (kept under the
    1024-descriptor SWDGE ring limit) + PE one-hot scatter-matmuls into PSUM
    (edges pre-sorted/packed by dst on host). Self-loop terms are NOT in the
    edge list: they are folded on DVE as agg += dinv2*H in the same op as the
    PSUM->SBUF copy (H1 kept in SBUF), cutting 20% of gather traffic.
  - mp1 gathers x as bf16 rows padded to 256B; layer matmuls and one-hot
    builds run at bf16 PE/DVE rates.
  - AllToAll (4 pipelined quarter-chunks, bf16) reshards the pre-W2
    aggregation to node-parallel: each core gets its 512-node slice for all
    32 batches, so each core streams only its 1/8 of advW/v1W (159MB bf16
    machine-wide, read once, unpadded 76-col tiles; first PRE_NB blocks
    prefetched into SBUF).
  - Head contraction is operand-swapped: the h2 slab [128k, 32sb] is the
    stationary operand (cheap weight loads) and headW streams; four k-slabs
    run concurrently in the PE's four 32-col groups (tile_position), with a
    single bank-wide dummy-clear then start=False accumulation (start=True
    clears has_written bank-wide).
  - AllReduce of [76,32] partial head sums; the tiny val-MLP and dueling
    combine run redundantly on every core; host takes core 0's output.
"""

import sys

sys.path.insert(0, "/opt/trn_rl_repo")

import os

import numpy as np
import ml_dtypes

# Precision mode: "f32" (exact), "bf16" (everything big in bf16), or a
# comma-set of {mp2,xfer,head}: mp2 = H1/messages/one-hots; xfer = A2A
# payload + W2; head = H2 + head weights. Accumulation is always fp32 PSUM.
PRECISION = os.environ.get("GCN_PREC", "bf16")
BF16 = np.dtype(ml_dtypes.bfloat16)


def _prec_groups():
    if PRECISION == "f32":
        return set()
    if PRECISION == "bf16":
        return {"mp2", "xfer", "head"}
    return set(PRECISION.split(","))


PREC_G = _prec_groups()

# ---------------- problem constants (hardcoded per contract) ----------------
B, N, F_IN, E = 32, 4096, 16, 16384
NC_CORES = 8
B_LOC = B // NC_CORES            # 4
NSLICE = N // NC_CORES           # 512 nodes per core for head phase
F1, F2 = 128, 256
P = 128
NTILES = N // P                  # 32 node tiles
BF1 = B_LOC * F_IN               # 64   (mp1 row width)
BFH = B_LOC * F1                 # 512  (H1 row width = mp2 gather width)
KTOT = NSLICE * F2               # 131072 contraction rows per core
KT = KTOT // P                   # 1024 K-tiles for head matmul
HW_W = 12 + 64                   # 76 head outputs (adv | v1)
# unpadded head-weight tiles: FWL would need 128 cols but the 68% extra
# HBM traffic costs more than the slower ldweights saves (DMA-bound kernel)
HW_P = HW_W
NT_HEAD = 16                     # nodes per W2/head block
PRE_NB = int(os.environ.get("GCN_PRE", "1"))   # head-weight nb-blocks in SBUF
# One dma_gather per node tile: a gather's descriptors must fit the 1024-slot
# SWDGE ring (single doorbell fires only after full emission — a gather
# bigger than the ring deadlocks on HW; the interpreter does not model this)
GSZ = int(os.environ.get("GCN_GSZ", "1"))
MSGBUF = int(os.environ.get("GCN_MSGBUF", "5"))  # msg/S pool depth
TILEPOS = os.environ.get("GCN_TILEPOS", "1") != "0"  # 4-way PE col tiling
N_CG = 4 if TILEPOS else 1       # PE col groups used by the head contraction
RHS_MERGE = os.environ.get("GCN_RHSMERGE", "1") != "0"  # single rhs-stage DMA
SRCSPLIT = os.environ.get("GCN_SRCSPLIT", "1") != "0"  # mp2 early/late gathers
A2A_EARLY = os.environ.get("GCN_A2AEARLY", "1") != "0"  # interleave A2A issue
# mp2 processes tiles quarter-major so A2A chunk q releases after 8 tiles
MP2_ORDER = [4 * k + q for q in range(4) for k in range(8)]


def _pack_sorted(src_a, dst_a, nrm_a, tile_order=None, src_split=None):
    """Sort edges by dst, pack into 128-edge chunks such that every chunk's
    dsts fall in one 128-node tile. Tiles are packed in `tile_order` so that
    the chunks of GSZ consecutive tiles in that order are contiguous (one
    dma_gather per tile group). With src_split, each tile's chunks are packed
    early-src-first (src < src_split) so the early gather can read a
    DRAM slice that is ready before the whole H1 is written; returns the
    per-tile early-chunk count in that case."""
    order = np.argsort(dst_a, kind="stable")
    src_a, dst_a, nrm_a = src_a[order], dst_a[order], nrm_a[order]

    src_pk, nrm_pk, off_pk = [], [], []
    chunk_tile = []
    n_early = {}
    for t in (tile_order if tile_order is not None else range(NTILES)):
        sel = (dst_a >= t * P) & (dst_a < (t + 1) * P)
        s, d, w = src_a[sel], dst_a[sel], nrm_a[sel]
        if src_split is not None:
            parts = []
            for half, hsel in ((0, s < src_split), (1, s >= src_split)):
                sh, dh, wh = s[hsel], d[hsel], w[hsel]
                cnt = len(sh)
                nch_h = (cnt + P - 1) // P
                pad = nch_h * P - cnt
                parts.append((
                    np.concatenate([sh, np.zeros(pad, np.int64)]),
                    np.concatenate([wh, np.zeros(pad, np.float32)]),
                    np.concatenate([dh - t * P, np.zeros(pad, np.int64)]),
                    nch_h))
            if parts[0][3] + parts[1][3] == 0:
                parts[0] = (np.zeros(P, np.int64), np.zeros(P, np.float32),
                            np.zeros(P, np.int64), 1)
            n_early[t] = parts[0][3]
            src_pk.extend([parts[0][0], parts[1][0]])
            nrm_pk.extend([parts[0][1], parts[1][1]])
            off_pk.extend([parts[0][2], parts[1][2]])
            chunk_tile.extend([t] * (parts[0][3] + parts[1][3]))
            continue
        cnt = len(s)
        nch = max(1, (cnt + P - 1) // P)
        pad = nch * P - cnt
        src_pk.append(np.concatenate([s, np.zeros(pad, np.int64)]))
        nrm_pk.append(np.concatenate([w, np.zeros(pad, np.float32)]))
        off_pk.append(np.concatenate([d - t * P, np.zeros(pad, np.int64)]))
        chunk_tile.extend([t] * nch)

    src_pk = np.concatenate(src_pk)
    nrm_pk = np.concatenate(nrm_pk)
    off_pk = np.concatenate(off_pk)
    e_pad = len(src_pk)
    nchunk = e_pad // P
    assert nchunk == len(chunk_tile)

    # dma_gather index table: logical idx i lives at [i % 16, i // 16]
    gidx = np.zeros((P, e_pad // 16), np.int16)
    for p16 in range(16):
        gidx[p16, :] = src_pk[p16::16].astype(np.int16)
    gidx = np.tile(gidx[:16], (8, 1))  # replicate over all 128 partitions

    # per-chunk column tables: [p, c] = value of edge c*128+p
    nrm_t = nrm_pk.reshape(nchunk, P).T.copy()          # [128, nchunk] f32
    off_t = off_pk.reshape(nchunk, P).T.astype(np.float32).copy()
    return gidx, nrm_t, off_t, chunk_tile, nchunk, n_early


def _pack_edges(edge_index, edge_weight):
    """Two packings: mp1 includes self-loop edges (x stays in DRAM only);
    mp2 excludes them — the self-loop term is folded on DVE from the
    SBUF-resident H1 (saves 20% of gather traffic and chunk matmuls).
    Also returns dinv2 [128, NTILES]: 1/deg for node t*128+p."""
    src = np.asarray(edge_index[0], np.int64)
    dst = np.asarray(edge_index[1], np.int64)
    ew = np.asarray(edge_weight, np.float32)

    deg = np.zeros(N, np.float32)
    np.add.at(deg, dst, ew)
    deg += 1.0
    dinv = (1.0 / np.sqrt(deg)).astype(np.float32)
    norm = ew * dinv[src] * dinv[dst]

    # mp1: edges + self loops (src=dst=n, weight 1/deg[n])
    src_a = np.concatenate([src, np.arange(N, dtype=np.int64)])
    dst_a = np.concatenate([dst, np.arange(N, dtype=np.int64)])
    nrm_a = np.concatenate([norm, dinv * dinv]).astype(np.float32)
    t1 = _pack_sorted(src_a, dst_a, nrm_a)
    # mp2: edges only, packed in mp2's quarter-major processing order; each
    # tile's chunks are early-src-first so the early gather only depends on
    # the first half of H1
    t2 = _pack_sorted(src, dst, norm.astype(np.float32), tile_order=MP2_ORDER,
                      src_split=(N // 2 if SRCSPLIT else None))

    dinv2_t = (dinv * dinv).reshape(NTILES, P).T.copy()  # [128, NTILES]
    return t1, t2, dinv2_t


def _prep_host(inputs):
    """All host-side numpy preprocessing: edge packing, weight layout, batch shard."""
    x = np.asarray(inputs["x"], np.float32)
    (gidx, nrm_t, off_t, chunk_tile, nchunk, _), \
        (gidx2, nrm_t2, off_t2, chunk_tile2, nchunk2, n_early2), dinv2_t = \
        _pack_edges(inputs["edge_index"], inputs["edge_weight"])

    W1 = np.asarray(inputs["W1"], np.float32)      # [16,128]
    b1 = np.asarray(inputs["b1"], np.float32)      # [128]
    W2 = np.asarray(inputs["W2"], np.float32)      # [128,256]
    b2 = np.asarray(inputs["b2"], np.float32)      # [256]
    advW = np.asarray(inputs["advW"], np.float32)  # [N*256, 12]
    advb = np.asarray(inputs["advb"], np.float32)
    v1W = np.asarray(inputs["v1W"], np.float32)    # [N*256, 64]
    v1b = np.asarray(inputs["v1b"], np.float32)
    v2W = np.asarray(inputs["v2W"], np.float32)
    v2b = np.asarray(inputs["v2b"], np.float32)
    v3W = np.asarray(inputs["v3W"], np.float32)
    v3b = np.asarray(inputs["v3b"], np.float32)

    # W1 block-diagonal over the 4 local batches, plus a bias row driven by
    # a constant-1 row appended to aggT on device: [65, 512]
    w1bd = np.zeros((BF1 + 1, B_LOC * F1), np.float32)
    for b in range(B_LOC):
        w1bd[b * F_IN:(b + 1) * F_IN, b * F1:(b + 1) * F1] = W1
    w1bd[BF1, :] = np.tile(b1, B_LOC)

    # dueling combine matrix (adv part): out = C.T @ adv + val
    C = np.zeros((12, 12), np.float32)
    for h in range(3):
        for a in range(4):
            i = h * 4 + a
            C[i, i] += 1.0
            for a2 in range(4):
                C[h * 4 + a2, i] -= 0.25

    shared = {
        "gidx": gidx,
        "nrm_t": nrm_t.copy(),
        "off_t": off_t.copy(),
        "gidx2": gidx2,
        "nrm_t2": nrm_t2.copy(),
        "off_t2": off_t2.copy(),
        "dinv2_t": dinv2_t.copy(),
        "w1bd": w1bd,
        "w2": (W2.astype(BF16) if "xfer" in PREC_G else W2).copy(),
        "b2c": b2[:, None].copy(),                  # [256,1]
        "advb_c": advb[:, None].copy(),             # [12,1]
        "v1b_c": v1b[:, None].copy(),               # [64,1]
        "v2w": v2W.copy(),                          # [64,64]
        "v2b_c": v2b[:, None].copy(),               # [64,1]
        "v3w": v3W.copy(),                          # [64,1]
        "v3b_c": v3b[None, :].copy(),               # [1,1]
        "cmat": C,
    }

    per_core = []
    for j in range(NC_CORES):
        # x batch-shard, node-major rows [N, b, f] -> [N, 64], bf16 padded to
        # 128 cols (gather elem_size_bytes must be a multiple of 256)
        x_nb = x[j * B_LOC:(j + 1) * B_LOC].transpose(1, 0, 2).reshape(N, BF1)
        x_loc = np.zeros((N, 2 * BF1), BF16)
        x_loc[:, :BF1] = x_nb.astype(BF16)
        # head weights: rows for this core's node slice, pre-tiled to
        # [128, KT*76]: col block j holds lhsT K-tile j = rows [128j,128j+128)
        r0 = j * KTOT
        aw = advW[r0:r0 + KTOT].reshape(KT, P, 12)
        vw = v1W[r0:r0 + KTOT].reshape(KT, P, 64)
        # v1 first (partitions 0:64), adv second (64:76): partition slices
        # must start at multiples of 32 on-device.
        hw = np.concatenate([vw, aw], axis=2)  # [KT, 128, 76]
        hw_t = hw.transpose(1, 0, 2).reshape(P, KT * HW_P)
        hw_t = (hw_t.astype(BF16) if "head" in PREC_G else hw_t).copy()
        per_core.append({"x_loc": x_loc, "headw_t": hw_t})

    return shared, per_core, (chunk_tile, chunk_tile2, n_early2), (nchunk, nchunk2)


# ---------------- device program ----------------

def build_program(nc, tc, chunk_tile, nchunk, io, collectives=True, phases=(1,1,1), repeat=1):
    """Emit the Tile program. io: dict of name -> DRAM AP."""
    import concourse.bass as bass
    import concourse.mybir as mybir
    import concourse.tile as tile
    from concourse.masks import make_identity

    f32 = mybir.dt.float32
    f32r = mybir.dt.float32r
    bf16 = mybir.dt.bfloat16
    mp2_dt = bf16 if "mp2" in PREC_G else f32
    xf_dt = bf16 if "xfer" in PREC_G else f32
    hd_dt = bf16 if "head" in PREC_G else f32
    i16 = mybir.dt.int16
    i32 = mybir.dt.int32
    AF = mybir.ActivationFunctionType
    OP = mybir.AluOpType

    chunk_tile1, chunk_tile2, n_early2 = chunk_tile
    nchunk1, nchunk2 = nchunk
    # chunks belonging to each node tile (contiguous ranges), per layer
    tile_chunks1 = [[] for _ in range(NTILES)]
    for c, t in enumerate(chunk_tile1):
        tile_chunks1[t].append(c)
    tile_chunks2 = [[] for _ in range(NTILES)]
    for c, t in enumerate(chunk_tile2):
        tile_chunks2[t].append(c)

    from contextlib import ExitStack
    with ExitStack() as ctx:
        const = ctx.enter_context(tc.tile_pool(name="const", bufs=1))
        sb = ctx.enter_context(tc.tile_pool(name="sb", bufs=3))
        sb_msg = ctx.enter_context(tc.tile_pool(name="msg", bufs=MSGBUF))
        sb_s = ctx.enter_context(tc.tile_pool(name="sbs", bufs=MSGBUF))
        sb_hw = ctx.enter_context(tc.tile_pool(name="sbhw", bufs=2))
        ps_agg = ctx.enter_context(tc.tile_pool(name="ps_agg", bufs=2, space="PSUM"))
        ps_t = ctx.enter_context(tc.tile_pool(name="ps_t", bufs=3, space="PSUM"))
        ps_tx = ctx.enter_context(tc.tile_pool(name="ps_tx", bufs=2, space="PSUM"))
        ps_head = ctx.enter_context(tc.tile_pool(name="ps_head", bufs=1, space="PSUM"))
        dram = ctx.enter_context(tc.tile_pool(name="dram", bufs=1, space="DRAM"))
        for _rep in range(repeat):
            # ---- constants into SBUF
            ident = const.tile([P, P], f32)
            make_identity(nc, ident[:])
            ident_x = const.tile([P, P], xf_dt)
            nc.vector.tensor_copy(ident_x[:], ident[:])
            iota_i = const.tile([P, P], i32)
            nc.gpsimd.iota(iota_i[:], pattern=[[1, P]], base=0, channel_multiplier=0)
            iota_f = const.tile([P, P], f32)
            nc.vector.tensor_copy(iota_f[:], iota_i[:])
            ones1 = const.tile([1, P], f32)
            nc.vector.memset(ones1[:], 1.0)

            gidx_sb = const.tile([P, nchunk1 * 8], i16)
            nc.sync.dma_start(gidx_sb[:], io["gidx"][:, :])
            nrm_sb = const.tile([P, nchunk1], f32)
            nc.sync.dma_start(nrm_sb[:], io["nrm_t"][:, :])
            off_sb = const.tile([P, nchunk1], f32)
            nc.sync.dma_start(off_sb[:], io["off_t"][:, :])
            gidx2_sb = const.tile([P, nchunk2 * 8], i16)
            nc.sync.dma_start(gidx2_sb[:], io["gidx2"][:, :])
            nrm2_sb = const.tile([P, nchunk2], f32)
            nc.sync.dma_start(nrm2_sb[:], io["nrm_t2"][:, :])
            off2_sb = const.tile([P, nchunk2], f32)
            nc.sync.dma_start(off2_sb[:], io["off_t2"][:, :])
            dinv2_sb = const.tile([P, NTILES], f32)
            nc.sync.dma_start(dinv2_sb[:], io["dinv2_t"][:, :])
            # SBUF-resident H1 (written by mp1, read by mp2's self-loop fold)
            h1_keep = const.tile([P, NTILES * BFH], mp2_dt)
            # head-weight prefetch: first PRE_NB nb-blocks' slabs (128*HW_P
            # cols each), loaded at kernel start to use DMA-idle time during
            # the PE-heavy GCN phase
            pre_cols = PRE_NB * 128 * HW_P
            hw_pre = const.tile([P, pre_cols], hd_dt)
            nc.sync.dma_start(hw_pre[:], io["headw_t"][:, 0:pre_cols])

            w1bd_sb = const.tile([BF1 + 1, B_LOC * F1], f32)
            nc.sync.dma_start(w1bd_sb[:], io["w1bd"][:, :])
            w2_sb = const.tile([P, F2], xf_dt)
            nc.sync.dma_start(w2_sb[:], io["w2"][:, :])
            # b2 [256,1] -> two [128,1] sbuf column stacks
            b2a = const.tile([P, 2], f32)
            nc.sync.dma_start(b2a[:, 0:1], io["b2c"][0:P, :])
            nc.sync.dma_start(b2a[:, 1:2], io["b2c"][P:F2, :])
            advb_sb = const.tile([12, 1], f32)
            nc.sync.dma_start(advb_sb[:], io["advb_c"][:, :])
            v1b_sb = const.tile([64, 1], f32)
            nc.sync.dma_start(v1b_sb[:], io["v1b_c"][:, :])
            v2w_sb = const.tile([64, 64], f32)
            nc.sync.dma_start(v2w_sb[:], io["v2w"][:, :])
            v2b_sb = const.tile([64, 1], f32)
            nc.sync.dma_start(v2b_sb[:], io["v2b_c"][:, :])
            v3w_sb = const.tile([64, 1], f32)
            nc.sync.dma_start(v3w_sb[:], io["v3w"][:, :])
            v3b_sb = const.tile([1, 1], f32)
            nc.sync.dma_start(v3b_sb[:], io["v3b_c"][:, :])
            cmat_sb = const.tile([12, 12], f32)
            nc.sync.dma_start(cmat_sb[:], io["cmat"][:, :])

            # scratch DRAM
            h1_dram = dram.tile([N, BFH], mp2_dt)        # node-major H1
            # agg2 feature-major, one buffer pair per node-quarter so the
            # AllToAll pipelines with mp2 and the head phase:
            # a2a_*_q[q][k, fin, n128, b]
            a2a_in_q = [dram.tile([NC_CORES, F1, P, B_LOC], xf_dt,
                                  name=f"a2ain{q}") for q in range(4)]
            a2a_out_q = [dram.tile([NC_CORES, F1, P, B_LOC], xf_dt,
                                   name=f"a2aout{q}") for q in range(4)]
            ar_in = dram.tile([HW_W, B], f32)
            ag_out = dram.tile([NC_CORES, HW_W, B], f32)

            # ================= mp1 + L1 feature matmul =================
            def mp_layer(x_dram, elem, out_cb, tiles, gi_sb, nr_sb, of_sb,
                         dt=f32, order=None, elem_u=None, early_split=None,
                         hooks=None):
                """gather + scatter for one GCN layer; out_cb(t, agg_psum_ap).

                One dma_gather covers GSZ consecutive tiles of the processing
                order (their chunks are packed contiguously). With
                early_split (maps tile -> early-chunk count), each tile's
                early chunks gather from the first half of x_dram only, so
                those gathers depend on half the producer writes and can
                hoist. hooks[i] is emitted after tile position i (used to
                interleave collective issue into this engine queue's program
                order). One-hot scatter matrices
                S[c][e, n] = norm[e] * (dstoff[e] == n) are built on DVE."""
                if elem_u is None:
                    elem_u = elem
                seq = list(order) if order is not None else list(range(NTILES))
                for gi in range(0, NTILES, GSZ):
                    gts = seq[gi:gi + GSZ]
                    g_cs = [c for t in gts for c in tiles[t]]
                    gc0, gnch = g_cs[0], len(g_cs)
                    assert g_cs == list(range(gc0, gc0 + gnch)), \
                        "group chunks must be contiguous (pack order mismatch)"
                    msg = sb_msg.tile([P, gnch * elem], dt, tag="msg")
                    msg3 = msg[:].rearrange("p (c e) -> p c e", e=elem)
                    if early_split is None:
                        nidx = gnch * P
                        nc.gpsimd.dma_gather(
                            out_ap=msg3,
                            in_ap=x_dram[:, :],
                            idxs_ap=gi_sb[:, gc0 * 8:(gc0 + gnch) * 8],
                            num_idxs=nidx,
                            num_idxs_reg=nidx,
                            elem_size=elem,
                        )
                    else:
                        assert GSZ == 1
                        t0 = gts[0]
                        ne = early_split[t0]
                        if ne > 0:
                            nc.gpsimd.dma_gather(
                                out_ap=msg3[:, 0:ne, :],
                                in_ap=x_dram[0:N // 2, :],
                                idxs_ap=gi_sb[:, gc0 * 8:(gc0 + ne) * 8],
                                num_idxs=ne * P,
                                num_idxs_reg=ne * P,
                                elem_size=elem,
                            )
                        if ne < gnch:
                            nc.gpsimd.dma_gather(
                                out_ap=msg3[:, ne:gnch, :],
                                in_ap=x_dram[:, :],
                                idxs_ap=gi_sb[:, (gc0 + ne) * 8:(gc0 + gnch) * 8],
                                num_idxs=(gnch - ne) * P,
                                num_idxs_reg=(gnch - ne) * P,
                                elem_size=elem,
                            )
                    for t in gts:
                        cs = tiles[t]
                        nch = len(cs)
                        s_t = sb_s.tile([P, nch * P], dt, tag="sC")
                        for i, c in enumerate(cs):
                            # S[e, n] = (iota[n] == dstoff[e]) * norm[e]
                            nc.vector.tensor_scalar(
                                out=s_t[:, i * P:(i + 1) * P], in0=iota_f[:],
                                scalar1=of_sb[:, c:c + 1], scalar2=nr_sb[:, c:c + 1],
                                op0=OP.is_equal, op1=OP.mult,
                            )
                        agg = ps_agg.tile([P, elem_u], f32, tag="agg")
                        for i, c in enumerate(cs):
                            mo = c - gc0
                            nc.tensor.matmul(
                                agg[:],
                                lhsT=s_t[:, i * P:(i + 1) * P],
                                rhs=msg[:, mo * elem:mo * elem + elem_u],
                                start=(i == 0),
                                stop=(i == nch - 1),
                            )
                        out_cb(t, agg)
                    if hooks is not None:
                        for pos in range(gi, gi + GSZ):
                            if pos in hooks:
                                hooks[pos]()

            # ---- layer 1
            def l1_out(t, agg):
                # transpose agg [128n, 64] -> aggT [64, 128n]
                agg_sb = sb.tile([P, BF1], f32, tag="agg1sb")
                nc.vector.tensor_copy(agg_sb[:], agg[:])
                psT = ps_t.tile([BF1, P], f32, tag="work")
                nc.tensor.transpose(psT[:], agg_sb[:], ident[:])
                aggT = sb.tile([BF1 + 1, P], f32, tag="aggT1")
                nc.vector.tensor_copy(aggT[0:BF1, :], psT[:])
                nc.vector.memset(aggT[BF1:BF1 + 1, :], 1.0)
                # H1[t] = relu(aggT_aug.T @ w1bd_aug)  (last row carries b1)
                psH = ps_t.tile([P, B_LOC * F1], f32, tag="work")
                nc.tensor.matmul(psH[:], lhsT=aggT[:], rhs=w1bd_sb[:], start=True, stop=True)
                h1slice = h1_keep[:, t * BFH:(t + 1) * BFH]
                nc.scalar.activation(h1slice, psH[:], AF.Relu)
                nc.sync.dma_start(h1_dram[t * P:(t + 1) * P, :], h1slice)

            if phases[0]:
                # x rows padded to 128 bf16 cols (256B gather min); matmuls
                # only consume the first 64
                mp_layer(io["x_loc"], 2 * BF1, l1_out, tile_chunks1,
                         gidx_sb, nrm_sb, off_sb, dt=bf16, elem_u=BF1)

            # ---- layer 2 message passing -> a2a_in (feature-major [k, fin, n', b]
            # so the post-A2A W2-rhs reads are contiguous per partition)
            def l2_out(t, agg):
                # agg_sb = agg + dinv2[t] * H1[t]  (self-loop fold, one DVE
                # op); output already in the A2A's bf16 — the payload is bf16
                # anyway, and bf16 PE transposes run 2x faster than f32
                agg_sb = sb.tile([P, BFH], xf_dt, tag="agg2sb")
                nc.vector.scalar_tensor_tensor(
                    out=agg_sb[:], in0=h1_keep[:, t * BFH:(t + 1) * BFH],
                    scalar=dinv2_sb[:, t:t + 1], in1=agg[:],
                    op0=OP.mult, op1=OP.add,
                )
                a2a_sb = sb.tile([P, BFH], xf_dt, tag="a2asb")
                a2a_3d = a2a_sb[:].rearrange("f (n b) -> f n b", b=B_LOC)
                for b in range(B_LOC):
                    psT = ps_tx.tile([P, P], xf_dt, tag="workx")
                    nc.tensor.transpose(psT[:], agg_sb[:, b * F1:(b + 1) * F1], ident_x[:])
                    nc.vector.tensor_copy(a2a_3d[:, :, b], psT[:])
                k, q = t // 4, t % 4
                nc.sync.dma_start(a2a_in_q[q][k, :, :, :], a2a_sb[:])

            def a2a_chunk(q):
                if collectives:
                    nc.gpsimd.collective_compute(
                        "AllToAll",
                        mybir.AluOpType.bypass,
                        replica_groups=[list(range(NC_CORES))],
                        ins=[a2a_in_q[q][:].opt()],
                        outs=[a2a_out_q[q][:].opt()],
                    )
                else:
                    for s in range(NC_CORES):
                        nc.sync.dma_start(
                            a2a_out_q[q][s].rearrange("f n b -> f (n b)"),
                            a2a_in_q[q][s].rearrange("f n b -> f (n b)"),
                        )

            if phases[1]:
                # quarter-major order: all of quarter q's tiles finish
                # together, releasing A2A chunk q while mp2 continues on
                # quarter q+1. The A2A issue is interleaved into the Pool
                # queue ~5 gathers into the NEXT quarter: by then quarter q's
                # compute is done, so the collective's input wait does not
                # head-of-line-block the remaining gathers (the transfer
                # itself runs async on the CCOM rings).
                hooks = None
                if A2A_EARLY:
                    hooks = {12: lambda: a2a_chunk(0),
                             20: lambda: a2a_chunk(1),
                             28: lambda: a2a_chunk(2),
                             31: lambda: a2a_chunk(3)}
                mp_layer(h1_dram, BFH, l2_out, tile_chunks2,
                         gidx2_sb, nrm2_sb, off2_sb, dt=mp2_dt,
                         order=MP2_ORDER,
                         early_split=(n_early2 if SRCSPLIT else None),
                         hooks=hooks)

            # ====== per-quarter AllToAll chunk + W2 + head partials ======
            # Head contraction runs operand-swapped: the h2 slab [128k, 32sb]
            # is the stationary operand (cheap 32-col weight loads) and the
            # headW slab [128k, 76] streams. Four consecutive k-slabs occupy
            # the PE's four 32-wide column groups (tile_position) and run
            # concurrently; ps_hd[32j:32j+32, :] holds col-group j's partial.
            NB_H = 64
            SB_COLS = NB_H * B_LOC                       # 256 cols per src core
            ps_hd = ps_head.tile([P, HW_W], f32)
            nblocks = (NSLICE // NB_H) if phases[2] else 0
            n_slab = nblocks * NB_H * 2
            slab_ctr = 0
            if not phases[2]:
                nc.vector.memset(ps_hd[:], 0.0)
            elif TILEPOS:
                # col-group accumulation shares one bank; start=True clears
                # has_written BANK-wide, so zero the whole region once with a
                # dummy matmul and accumulate everything else with start=False
                zero_w = const.tile([P, P], f32)
                nc.vector.memset(zero_w[:], 0.0)
                nc.tensor.matmul(ps_hd[:], lhsT=zero_w[:],
                                 rhs=ident[:, 0:HW_W], start=True, stop=False,
                                 skip_group_check=True)

            for nb in range(nblocks):
                q, half = nb // 2, nb % 2
                if half == 0 and not (A2A_EARLY and phases[1]):
                    a2a_chunk(q)
                # stage rhs [128 fin, (s, n, b)]: 512B runs per partition
                rhs_sb = sb.tile([P, NC_CORES * SB_COLS], xf_dt, tag="w2rhs")
                if RHS_MERGE:
                    nc.sync.dma_start(
                        rhs_sb[:].rearrange("p (s n b) -> p s n b",
                                            s=NC_CORES, n=NB_H, b=B_LOC),
                        a2a_out_q[q][:, :, half * NB_H:(half + 1) * NB_H, :]
                        .rearrange("s f n b -> f s n b"),
                    )
                else:
                    for s in range(NC_CORES):
                        nc.sync.dma_start(
                            rhs_sb[:, s * SB_COLS:(s + 1) * SB_COLS],
                            a2a_out_q[q][s, :, half * NB_H:(half + 1) * NB_H, :],
                        )
                h2 = []
                for fh in range(2):
                    # h2 stored node-major (n, s, b) so the head lhsT slab
                    # [128, 32] for node i is a contiguous column slice
                    h2sb = sb.tile([P, NC_CORES * SB_COLS], hd_dt, tag="h2sb")
                    h2v = h2sb[:].rearrange("p (n s b) -> p n s b",
                                            n=NB_H, s=NC_CORES, b=B_LOC)
                    for qq in range(4):  # free split: 512-col matmuls
                        sl = slice(qq * 512, (qq + 1) * 512)
                        psW = ps_t.tile([P, 512], f32, tag="work")
                        nc.tensor.matmul(
                            psW[:], lhsT=w2_sb[:, fh * P:(fh + 1) * P],
                            rhs=rhs_sb[:, sl], start=True, stop=True,
                        )
                        # relu + per-partition bias b2[fh*128 + p]; psW's
                        # column order is (s, n, b) for s in {2qq, 2qq+1}
                        nc.scalar.activation(
                            h2v[:, :, 2 * qq:2 * qq + 2, :]
                            .rearrange("p n s b -> p s n b"),
                            psW[:].rearrange("p (s n b) -> p s n b",
                                             s=2, n=NB_H, b=B_LOC),
                            AF.Relu, bias=b2a[:, fh:fh + 1])
                    h2.append(h2sb)
                hw_sb = None
                for i in range(NB_H):
                    g = (nb * NB_H + i) // 16          # 16-node headW slab index
                    if nb < PRE_NB:
                        hw_view = hw_pre[:, g * 32 * HW_P:(g + 1) * 32 * HW_P]
                    else:
                        if i % 16 == 0:
                            hw_sb = sb_hw.tile([P, 32 * HW_P], hd_dt, tag="hwslab")
                            nc.sync.dma_start(
                                hw_sb[:],
                                io["headw_t"][:, g * 32 * HW_P:(g + 1) * 32 * HW_P])
                        hw_view = hw_sb[:]
                    for fh in range(2):
                        jj = 2 * (i % 16) + fh
                        j4 = slab_ctr % N_CG
                        nc.tensor.matmul(
                            ps_hd[32 * j4:32 * (j4 + 1), :],
                            lhsT=h2[fh][:, i * 32:(i + 1) * 32],
                            rhs=hw_view[:, jj * HW_P:(jj + 1) * HW_P],
                            start=(not TILEPOS) and slab_ctr == 0,
                            stop=(slab_ctr >= n_slab - N_CG),
                            tile_position=(0, 32 * j4) if TILEPOS else None,
                            skip_group_check=True,
                        )
                        slab_ctr += 1

            # ps_hd [N_CG*32b, 76] -> transpose -> [76, N_CG*32b] -> sum groups
            hd_sb = sb.tile([32 * N_CG, HW_W], f32, tag="hdsb")
            nc.vector.tensor_copy(hd_sb[:], ps_hd[0:32 * N_CG, :])
            psTr = ps_t.tile([HW_W, 32 * N_CG], f32, tag="work")
            nc.tensor.transpose(psTr[:], hd_sb[:],
                                ident[0:32 * N_CG, 0:32 * N_CG])
            tr_sb = sb.tile([HW_W, 32 * N_CG], f32, tag="trsb")
            nc.vector.tensor_copy(tr_sb[:], psTr[:])
            if N_CG == 4:
                hp0 = sb.tile([HW_W, 2 * B], f32, tag="hp0")
                nc.vector.scalar_tensor_tensor(
                    out=hp0[:, 0:B], in0=tr_sb[:, 0:B], scalar=1.0,
                    in1=tr_sb[:, B:2 * B], op0=OP.mult, op1=OP.add)
                nc.vector.scalar_tensor_tensor(
                    out=hp0[:, B:2 * B], in0=tr_sb[:, 2 * B:3 * B], scalar=1.0,
                    in1=tr_sb[:, 3 * B:4 * B], op0=OP.mult, op1=OP.add)
                part_sb = sb.tile([HW_W, B], f32, tag="part")
                nc.vector.scalar_tensor_tensor(
                    out=part_sb[:], in0=hp0[:, 0:B], scalar=1.0,
                    in1=hp0[:, B:2 * B], op0=OP.mult, op1=OP.add)
            else:
                part_sb = tr_sb
            nc.sync.dma_start(ar_in[:, :], part_sb[:, 0:B])

            # ============ AllGather partials + on-chip tree sum ============
            # (AllGather + 7 DVE adds beats AllReduce for a 9.7KB payload:
            # no reduce phase, ~half the collective latency)
            if collectives:
                nc.gpsimd.collective_compute(
                    "AllGather",
                    mybir.AluOpType.bypass,
                    replica_groups=[list(range(NC_CORES))],
                    ins=[ar_in[:].opt()],
                    outs=[ag_out[:].opt()],
                )
            else:
                for s in range(NC_CORES):
                    nc.sync.dma_start(ag_out[s, :, :], ar_in[:, :])
            red8_sb = sb.tile([HW_W, NC_CORES * B], f32, tag="red8")
            nc.sync.dma_start(
                red8_sb[:].rearrange("h (s b) -> h s b", s=NC_CORES, b=B),
                ag_out[:].rearrange("s h b -> h s b"),
            )
            for j in range(4):
                nc.vector.scalar_tensor_tensor(
                    out=red8_sb[:, j * B:(j + 1) * B],
                    in0=red8_sb[:, j * B:(j + 1) * B], scalar=1.0,
                    in1=red8_sb[:, (j + 4) * B:(j + 5) * B],
                    op0=OP.mult, op1=OP.add)
            for j in range(2):
                nc.vector.scalar_tensor_tensor(
                    out=red8_sb[:, j * B:(j + 1) * B],
                    in0=red8_sb[:, j * B:(j + 1) * B], scalar=1.0,
                    in1=red8_sb[:, (j + 2) * B:(j + 3) * B],
                    op0=OP.mult, op1=OP.add)
            red_sb = sb.tile([HW_W, B], f32, tag="red")
            nc.vector.scalar_tensor_tensor(
                out=red_sb[:], in0=red8_sb[:, 0:B], scalar=1.0,
                in1=red8_sb[:, B:2 * B], op0=OP.mult, op1=OP.add)

            # ================= final MLP + dueling combine =================
            adv_sb = sb.tile([12, B], f32, tag="adv")
            nc.scalar.activation(adv_sb[:], red_sb[64:76, :], AF.Relu, bias=advb_sb[:])
            # val path
            v1_sb = sb.tile([64, B], f32, tag="v1")
            nc.scalar.activation(v1_sb[:], red_sb[0:64, :], AF.Relu, bias=v1b_sb[:])
            psV = ps_t.tile([64, B], f32, tag="work")
            nc.tensor.matmul(psV[:], lhsT=v2w_sb[:], rhs=v1_sb[:], start=True, stop=True)
            v2_sb = sb.tile([64, B], f32, tag="v2")
            nc.scalar.activation(v2_sb[:], psV[:], AF.Relu, bias=v2b_sb[:])
            psV3 = ps_t.tile([1, B], f32, tag="work")
            nc.tensor.matmul(psV3[:], lhsT=v3w_sb[:], rhs=v2_sb[:], start=True, stop=True)
            val_sb = sb.tile([1, B], f32, tag="val")
            nc.vector.tensor_scalar_add(val_sb[:], psV3[:], v3b_sb[0:1, 0:1])
            # out = cmat.T @ adv + 1.T @ val
            psO = ps_t.tile([12, B], f32, tag="work")
            nc.tensor.matmul(psO[:], lhsT=cmat_sb[:], rhs=adv_sb[:], start=True, stop=False)
            nc.tensor.matmul(psO[:], lhsT=ones1[:, 0:12], rhs=val_sb[:], start=False, stop=True)
            out_sb = sb.tile([12, B], f32, tag="out")
            nc.vector.tensor_copy(out_sb[:], psO[:])
            nc.sync.dma_start(io["out"][:, :], out_sb[:])


# ---------------- driver ----------------

LAST_RESULTS = None

def _input_specs(shared, per_core):
    """name -> (shape, np dtype); per-core entries use per_core[0] shapes."""
    specs = {}
    for k, v in shared.items():
        specs[k] = v
    for k, v in per_core[0].items():
        specs[k] = v
    return specs


def kernel(**inputs) -> np.ndarray:
    import concourse.bacc as bacc
    import concourse.mybir as mybir
    import concourse.tile as tile
    from concourse import bass_utils

    shared, per_core, chunk_tile, nchunk = _prep_host(inputs)

    nc = bacc.Bacc("TRN2", target_bir_lowering=False, debug=False,
                   enable_asserts=False, num_devices=NC_CORES)

    io = {}
    specs = _input_specs(shared, per_core)
    for name, arr in specs.items():
        io[name] = nc.dram_tensor(
            name, list(arr.shape), mybir.dt.from_np(arr.dtype), kind="ExternalInput"
        ).ap()
    io["out"] = nc.dram_tensor(
        "out", [12, B], mybir.dt.float32, kind="ExternalOutput"
    ).ap()

    with tile.TileContext(nc) as tc:
        build_program(nc, tc, chunk_tile, nchunk, io)
    nc.compile()

    in_maps = []
    for j in range(NC_CORES):
        m = dict(shared)
        m.update(per_core[j])
        in_maps.append(m)

    res = bass_utils.run_bass_kernel_spmd(
        nc, in_maps, core_ids=list(range(NC_CORES)),
    )
    global LAST_RESULTS
    LAST_RESULTS = res
    out = res.results[0]["out"]                      # [12, 32]
    return out.T.reshape(B, 3, 4).copy().astype(np.float32)


if __name__ == "__main__":
    rng = np.random.default_rng(0)
    ei = rng.integers(0, N, (2, E)).astype(np.int64)
    demo = {
        "x": rng.standard_normal((B, N, F_IN), np.float32),
        "edge_index": ei,
        "edge_weight": rng.random(E, np.float32),
        "W1": rng.standard_normal((F_IN, F1), np.float32) / 4,
        "b1": np.zeros(F1, np.float32),
        "W2": rng.standard_normal((F1, F2), np.float32) / 11.3,
        "b2": np.zeros(F2, np.float32),
        "advW": rng.standard_normal((N * F2, 12), np.float32) / 1024,
        "advb": np.zeros(12, np.float32),
        "v1W": rng.standard_normal((N * F2, 64), np.float32) / 1024,
        "v1b": np.zeros(64, np.float32),
        "v2W": rng.standard_normal((64, 64), np.float32) / 8,
        "v2b": np.zeros(64, np.float32),
        "v3W": rng.standard_normal((64, 1), np.float32) / 8,
        "v3b": np.zeros(1, np.float32),
    }
    print(kernel(**demo).shape)



# revision 71
# speedup vs baseline: 1.2244x; 1.2244x over previous
"""Trainium2 Bass kernel for BHS_GCN: 2x GCNConv + dueling value/advantage heads.

Strategy (8 NeuronCores, single NEFF launch, bf16 compute / fp32 PSUM):
  - GCN phase batch-parallel: each core owns B_LOC=4 full graphs.
    Message passing = per-tile dma_gather of source-node rows (kept under the
    1024-descriptor SWDGE ring limit) + PE one-hot scatter-matmuls into PSUM
    (edges pre-sorted/packed by dst on host). Self-loop terms are NOT in the
    edge list: they are folded on DVE as agg += dinv2*H in the same op as the
    PSUM->SBUF copy (H1 kept in SBUF), cutting 20% of gather traffic.
  - mp1 gathers x as bf16 rows padded to 256B; layer matmuls and one-hot
    builds run at bf16 PE/DVE rates.
  - AllToAll (4 pipelined quarter-chunks, bf16) reshards the pre-W2
    aggregation to node-parallel: each core gets its 512-node slice for all
    32 batches, so each core streams only its 1/8 of advW/v1W (159MB bf16
    machine-wide, read once, unpadded 76-col tiles; first PRE_NB blocks
    prefetched into SBUF).
  - Head contraction is operand-swapped: the h2 slab [128k, 32sb] is the
    stationary operand (cheap weight loads) and headW streams; four k-slabs
    run concurrently in the PE's four 32-col groups (tile_position), with a
    single bank-wide dummy-clear then start=False accumulation (start=True
    clears has_written bank-wide).
  - AllReduce of [76,32] partial head sums; the tiny val-MLP and dueling
    combine run redundantly on every core; host takes core 0's output.
"""

import sys

sys.path.insert(0, "/opt/trn_rl_repo")

import os

import numpy as np
import ml_dtypes

# Precision mode: "f32" (exact), "bf16" (everything big in bf16), or a
# comma-set of {mp2,xfer,head}: mp2 = H1/messages/one-hots; xfer = A2A
# payload + W2; head = H2 + head weights. Accumulation is always fp32 PSUM.
PRECISION = os.environ.get("GCN_PREC", "bf16")
BF16 = np.dtype(ml_dtypes.bfloat16)


def _prec_groups():
    if PRECISION == "f32":
        return set()
    if PRECISION == "bf16":
        return {"mp2", "xfer", "head"}
    return set(PRECISION.split(","))


PREC_G = _prec_groups()

# ---------------- problem constants (hardcoded per contract) ----------------
B, N, F_IN, E = 32, 4096, 16, 16384
NC_CORES = 8
B_LOC = B // NC_CORES            # 4
NSLICE = N // NC_CORES           # 512 nodes per core for head phase
F1, F2 = 128, 256
P = 128
NTILES = N // P                  # 32 node tiles
BF1 = B_LOC * F_IN               # 64   (mp1 row width)
BFH = B_LOC * F1                 # 512  (H1 row width = mp2 gather width)
KTOT = NSLICE * F2               # 131072 contraction rows per core
KT = KTOT // P                   # 1024 K-tiles for head matmul
HW_W = 12 + 64                   # 76 head outputs (adv | v1)
# unpadded head-weight tiles: FWL would need 128 cols but the 68% extra
# HBM traffic costs more than the slower ldweights saves (DMA-bound kernel)
HW_P = HW_W
NT_HEAD = 16                     # nodes per W2/head block
PRE_NB = int(os.environ.get("GCN_PRE", "1"))   # head-weight nb-blocks in SBUF
# One dma_gather per node tile: a gather's descriptors must fit the 1024-slot
# SWDGE ring (single doorbell fires only after full emission — a gather
# bigger than the ring deadlocks on HW; the interpreter does not model this)
GSZ = int(os.environ.get("GCN_GSZ", "1"))
MSGBUF = int(os.environ.get("GCN_MSGBUF", "5"))  # msg/S pool depth
TILEPOS = os.environ.get("GCN_TILEPOS", "1") != "0"  # 4-way PE col tiling
N_CG = 4 if TILEPOS else 1       # PE col groups used by the head contraction
RHS_MERGE = os.environ.get("GCN_RHSMERGE", "1") != "0"  # single rhs-stage DMA
SRCSPLIT = os.environ.get("GCN_SRCSPLIT", "1") != "0"  # mp2 early/late gathers
# Interleaving A2A issue among the gathers REGRESSES on HW: the collective
# instruction occupies the GPSIMD queue for its whole transfer, stalling the
# gathers queued behind it. Keep collectives after all gathers (default off).
A2A_EARLY = os.environ.get("GCN_A2AEARLY", "0") != "0"
# mp2 processes tiles quarter-major so A2A chunk q releases after 8 tiles
MP2_ORDER = [4 * k + q for q in range(4) for k in range(8)]


def _pack_sorted(src_a, dst_a, nrm_a, tile_order=None, src_split=None):
    """Sort edges by dst, pack into 128-edge chunks such that every chunk's
    dsts fall in one 128-node tile. Tiles are packed in `tile_order` so that
    the chunks of GSZ consecutive tiles in that order are contiguous (one
    dma_gather per tile group). With src_split, each tile's chunks are packed
    early-src-first (src < src_split) so the early gather can read a
    DRAM slice that is ready before the whole H1 is written; returns the
    per-tile early-chunk count in that case."""
    order = np.argsort(dst_a, kind="stable")
    src_a, dst_a, nrm_a = src_a[order], dst_a[order], nrm_a[order]

    src_pk, nrm_pk, off_pk = [], [], []
    chunk_tile = []
    n_early = {}
    for t in (tile_order if tile_order is not None else range(NTILES)):
        sel = (dst_a >= t * P) & (dst_a < (t + 1) * P)
        s, d, w = src_a[sel], dst_a[sel], nrm_a[sel]
        if src_split is not None:
            parts = []
            for half, hsel in ((0, s < src_split), (1, s >= src_split)):
                sh, dh, wh = s[hsel], d[hsel], w[hsel]
                cnt = len(sh)
                nch_h = (cnt + P - 1) // P
                pad = nch_h * P - cnt
                parts.append((
                    np.concatenate([sh, np.zeros(pad, np.int64)]),
                    np.concatenate([wh, np.zeros(pad, np.float32)]),
                    np.concatenate([dh - t * P, np.zeros(pad, np.int64)]),
                    nch_h))
            if parts[0][3] + parts[1][3] == 0:
                parts[0] = (np.zeros(P, np.int64), np.zeros(P, np.float32),
                            np.zeros(P, np.int64), 1)
            n_early[t] = parts[0][3]
            src_pk.extend([parts[0][0], parts[1][0]])
            nrm_pk.extend([parts[0][1], parts[1][1]])
            off_pk.extend([parts[0][2], parts[1][2]])
            chunk_tile.extend([t] * (parts[0][3] + parts[1][3]))
            continue
        cnt = len(s)
        nch = max(1, (cnt + P - 1) // P)
        pad = nch * P - cnt
        src_pk.append(np.concatenate([s, np.zeros(pad, np.int64)]))
        nrm_pk.append(np.concatenate([w, np.zeros(pad, np.float32)]))
        off_pk.append(np.concatenate([d - t * P, np.zeros(pad, np.int64)]))
        chunk_tile.extend([t] * nch)

    src_pk = np.concatenate(src_pk)
    nrm_pk = np.concatenate(nrm_pk)
    off_pk = np.concatenate(off_pk)
    e_pad = len(src_pk)
    nchunk = e_pad // P
    assert nchunk == len(chunk_tile)

    # dma_gather index table: logical idx i lives at [i % 16, i // 16]
    gidx = np.zeros((P, e_pad // 16), np.int16)
    for p16 in range(16):
        gidx[p16, :] = src_pk[p16::16].astype(np.int16)
    gidx = np.tile(gidx[:16], (8, 1))  # replicate over all 128 partitions

    # per-chunk column tables: [p, c] = value of edge c*128+p
    nrm_t = nrm_pk.reshape(nchunk, P).T.copy()          # [128, nchunk] f32
    off_t = off_pk.reshape(nchunk, P).T.astype(np.float32).copy()
    return gidx, nrm_t, off_t, chunk_tile, nchunk, n_early


def _pack_edges(edge_index, edge_weight):
    """Two packings: mp1 includes self-loop edges (x stays in DRAM only);
    mp2 excludes them — the self-loop term is folded on DVE from the
    SBUF-resident H1 (saves 20% of gather traffic and chunk matmuls).
    Also returns dinv2 [128, NTILES]: 1/deg for node t*128+p."""
    src = np.asarray(edge_index[0], np.int64)
    dst = np.asarray(edge_index[1], np.int64)
    ew = np.asarray(edge_weight, np.float32)

    deg = np.zeros(N, np.float32)
    np.add.at(deg, dst, ew)
    deg += 1.0
    dinv = (1.0 / np.sqrt(deg)).astype(np.float32)
    norm = ew * dinv[src] * dinv[dst]

    # mp1: edges + self loops (src=dst=n, weight 1/deg[n])
    src_a = np.concatenate([src, np.arange(N, dtype=np.int64)])
    dst_a = np.concatenate([dst, np.arange(N, dtype=np.int64)])
    nrm_a = np.concatenate([norm, dinv * dinv]).astype(np.float32)
    t1 = _pack_sorted(src_a, dst_a, nrm_a)
    # mp2: edges only, packed in mp2's quarter-major processing order; each
    # tile's chunks are early-src-first so the early gather only depends on
    # the first half of H1
    t2 = _pack_sorted(src, dst, norm.astype(np.float32), tile_order=MP2_ORDER,
                      src_split=(N // 2 if SRCSPLIT else None))

    dinv2_t = (dinv * dinv).reshape(NTILES, P).T.copy()  # [128, NTILES]
    return t1, t2, dinv2_t


def _prep_host(inputs):
    """All host-side numpy preprocessing: edge packing, weight layout, batch shard."""
    x = np.asarray(inputs["x"], np.float32)
    (gidx, nrm_t, off_t, chunk_tile, nchunk, _), \
        (gidx2, nrm_t2, off_t2, chunk_tile2, nchunk2, n_early2), dinv2_t = \
        _pack_edges(inputs["edge_index"], inputs["edge_weight"])

    W1 = np.asarray(inputs["W1"], np.float32)      # [16,128]
    b1 = np.asarray(inputs["b1"], np.float32)      # [128]
    W2 = np.asarray(inputs["W2"], np.float32)      # [128,256]
    b2 = np.asarray(inputs["b2"], np.float32)      # [256]
    advW = np.asarray(inputs["advW"], np.float32)  # [N*256, 12]
    advb = np.asarray(inputs["advb"], np.float32)
    v1W = np.asarray(inputs["v1W"], np.float32)    # [N*256, 64]
    v1b = np.asarray(inputs["v1b"], np.float32)
    v2W = np.asarray(inputs["v2W"], np.float32)
    v2b = np.asarray(inputs["v2b"], np.float32)
    v3W = np.asarray(inputs["v3W"], np.float32)
    v3b = np.asarray(inputs["v3b"], np.float32)

    # W1 block-diagonal over the 4 local batches, plus a bias row driven by
    # a constant-1 row appended to aggT on device: [65, 512]
    w1bd = np.zeros((BF1 + 1, B_LOC * F1), np.float32)
    for b in range(B_LOC):
        w1bd[b * F_IN:(b + 1) * F_IN, b * F1:(b + 1) * F1] = W1
    w1bd[BF1, :] = np.tile(b1, B_LOC)

    # dueling combine matrix (adv part): out = C.T @ adv + val
    C = np.zeros((12, 12), np.float32)
    for h in range(3):
        for a in range(4):
            i = h * 4 + a
            C[i, i] += 1.0
            for a2 in range(4):
                C[h * 4 + a2, i] -= 0.25

    shared = {
        "gidx": gidx,
        "nrm_t": nrm_t.copy(),
        "off_t": off_t.copy(),
        "gidx2": gidx2,
        "nrm_t2": nrm_t2.copy(),
        "off_t2": off_t2.copy(),
        "dinv2_t": dinv2_t.copy(),
        "w1bd": w1bd,
        "w2": (W2.astype(BF16) if "xfer" in PREC_G else W2).copy(),
        "b2c": b2[:, None].copy(),                  # [256,1]
        "advb_c": advb[:, None].copy(),             # [12,1]
        "v1b_c": v1b[:, None].copy(),               # [64,1]
        "v2w": v2W.copy(),                          # [64,64]
        "v2b_c": v2b[:, None].copy(),               # [64,1]
        "v3w": v3W.copy(),                          # [64,1]
        "v3b_c": v3b[None, :].copy(),               # [1,1]
        "cmat": C,
    }

    per_core = []
    for j in range(NC_CORES):
        # x batch-shard, node-major rows [N, b, f] -> [N, 64], bf16 padded to
        # 128 cols (gather elem_size_bytes must be a multiple of 256)
        x_nb = x[j * B_LOC:(j + 1) * B_LOC].transpose(1, 0, 2).reshape(N, BF1)
        x_loc = np.zeros((N, 2 * BF1), BF16)
        x_loc[:, :BF1] = x_nb.astype(BF16)
        # head weights: rows for this core's node slice, pre-tiled to
        # [128, KT*76]: col block j holds lhsT K-tile j = rows [128j,128j+128)
        r0 = j * KTOT
        aw = advW[r0:r0 + KTOT].reshape(KT, P, 12)
        vw = v1W[r0:r0 + KTOT].reshape(KT, P, 64)
        # v1 first (partitions 0:64), adv second (64:76): partition slices
        # must start at multiples of 32 on-device.
        hw = np.concatenate([vw, aw], axis=2)  # [KT, 128, 76]
        hw_t = hw.transpose(1, 0, 2).reshape(P, KT * HW_P)
        hw_t = (hw_t.astype(BF16) if "head" in PREC_G else hw_t).copy()
        per_core.append({"x_loc": x_loc, "headw_t": hw_t})

    return shared, per_core, (chunk_tile, chunk_tile2, n_early2), (nchunk, nchunk2)


# ---------------- device program ----------------

def build_program(nc, tc, chunk_tile, nchunk, io, collectives=True, phases=(1,1,1), repeat=1):
    """Emit the Tile program. io: dict of name -> DRAM AP."""
    import concourse.bass as bass
    import concourse.mybir as mybir
    import concourse.tile as tile
    from concourse.masks import make_identity

    f32 = mybir.dt.float32
    f32r = mybir.dt.float32r
    bf16 = mybir.dt.bfloat16
    mp2_dt = bf16 if "mp2" in PREC_G else f32
    xf_dt = bf16 if "xfer" in PREC_G else f32
    hd_dt = bf16 if "head" in PREC_G else f32
    i16 = mybir.dt.int16
    i32 = mybir.dt.int32
    AF = mybir.ActivationFunctionType
    OP = mybir.AluOpType

    chunk_tile1, chunk_tile2, n_early2 = chunk_tile
    nchunk1, nchunk2 = nchunk
    # chunks belonging to each node tile (contiguous ranges), per layer
    tile_chunks1 = [[] for _ in range(NTILES)]
    for c, t in enumerate(chunk_tile1):
        tile_chunks1[t].append(c)
    tile_chunks2 = [[] for _ in range(NTILES)]
    for c, t in enumerate(chunk_tile2):
        tile_chunks2[t].append(c)

    from contextlib import ExitStack
    with ExitStack() as ctx:
        const = ctx.enter_context(tc.tile_pool(name="const", bufs=1))
        sb = ctx.enter_context(tc.tile_pool(name="sb", bufs=3))
        sb_msg = ctx.enter_context(tc.tile_pool(name="msg", bufs=MSGBUF))
        sb_s = ctx.enter_context(tc.tile_pool(name="sbs", bufs=MSGBUF))
        sb_hw = ctx.enter_context(tc.tile_pool(name="sbhw", bufs=2))
        ps_agg = ctx.enter_context(tc.tile_pool(name="ps_agg", bufs=2, space="PSUM"))
        ps_t = ctx.enter_context(tc.tile_pool(name="ps_t", bufs=3, space="PSUM"))
        ps_tx = ctx.enter_context(tc.tile_pool(name="ps_tx", bufs=2, space="PSUM"))
        ps_head = ctx.enter_context(tc.tile_pool(name="ps_head", bufs=1, space="PSUM"))
        dram = ctx.enter_context(tc.tile_pool(name="dram", bufs=1, space="DRAM"))
        for _rep in range(repeat):
            # ---- constants into SBUF
            ident = const.tile([P, P], f32)
            make_identity(nc, ident[:])
            ident_x = const.tile([P, P], xf_dt)
            nc.vector.tensor_copy(ident_x[:], ident[:])
            iota_i = const.tile([P, P], i32)
            nc.gpsimd.iota(iota_i[:], pattern=[[1, P]], base=0, channel_multiplier=0)
            iota_f = const.tile([P, P], f32)
            nc.vector.tensor_copy(iota_f[:], iota_i[:])
            ones1 = const.tile([1, P], f32)
            nc.vector.memset(ones1[:], 1.0)

            gidx_sb = const.tile([P, nchunk1 * 8], i16)
            nc.sync.dma_start(gidx_sb[:], io["gidx"][:, :])
            nrm_sb = const.tile([P, nchunk1], f32)
            nc.sync.dma_start(nrm_sb[:], io["nrm_t"][:, :])
            off_sb = const.tile([P, nchunk1], f32)
            nc.sync.dma_start(off_sb[:], io["off_t"][:, :])
            gidx2_sb = const.tile([P, nchunk2 * 8], i16)
            nc.sync.dma_start(gidx2_sb[:], io["gidx2"][:, :])
            nrm2_sb = const.tile([P, nchunk2], f32)
            nc.sync.dma_start(nrm2_sb[:], io["nrm_t2"][:, :])
            off2_sb = const.tile([P, nchunk2], f32)
            nc.sync.dma_start(off2_sb[:], io["off_t2"][:, :])
            dinv2_sb = const.tile([P, NTILES], f32)
            nc.sync.dma_start(dinv2_sb[:], io["dinv2_t"][:, :])
            # SBUF-resident H1 (written by mp1, read by mp2's self-loop fold)
            h1_keep = const.tile([P, NTILES * BFH], mp2_dt)
            # head-weight prefetch: first PRE_NB nb-blocks' slabs (128*HW_P
            # cols each), loaded at kernel start to use DMA-idle time during
            # the PE-heavy GCN phase
            pre_cols = PRE_NB * 128 * HW_P
            hw_pre = const.tile([P, pre_cols], hd_dt)
            nc.sync.dma_start(hw_pre[:], io["headw_t"][:, 0:pre_cols])

            w1bd_sb = const.tile([BF1 + 1, B_LOC * F1], f32)
            nc.sync.dma_start(w1bd_sb[:], io["w1bd"][:, :])
            w2_sb = const.tile([P, F2], xf_dt)
            nc.sync.dma_start(w2_sb[:], io["w2"][:, :])
            # b2 [256,1] -> two [128,1] sbuf column stacks
            b2a = const.tile([P, 2], f32)
            nc.sync.dma_start(b2a[:, 0:1], io["b2c"][0:P, :])
            nc.sync.dma_start(b2a[:, 1:2], io["b2c"][P:F2, :])
            advb_sb = const.tile([12, 1], f32)
            nc.sync.dma_start(advb_sb[:], io["advb_c"][:, :])
            v1b_sb = const.tile([64, 1], f32)
            nc.sync.dma_start(v1b_sb[:], io["v1b_c"][:, :])
            v2w_sb = const.tile([64, 64], f32)
            nc.sync.dma_start(v2w_sb[:], io["v2w"][:, :])
            v2b_sb = const.tile([64, 1], f32)
            nc.sync.dma_start(v2b_sb[:], io["v2b_c"][:, :])
            v3w_sb = const.tile([64, 1], f32)
            nc.sync.dma_start(v3w_sb[:], io["v3w"][:, :])
            v3b_sb = const.tile([1, 1], f32)
            nc.sync.dma_start(v3b_sb[:], io["v3b_c"][:, :])
            cmat_sb = const.tile([12, 12], f32)
            nc.sync.dma_start(cmat_sb[:], io["cmat"][:, :])

            # scratch DRAM
            h1_dram = dram.tile([N, BFH], mp2_dt)        # node-major H1
            # agg2 feature-major, one buffer pair per node-quarter so the
            # AllToAll pipelines with mp2 and the head phase:
            # a2a_*_q[q][k, fin, n128, b]
            a2a_in_q = [dram.tile([NC_CORES, F1, P, B_LOC], xf_dt,
                                  name=f"a2ain{q}") for q in range(4)]
            a2a_out_q = [dram.tile([NC_CORES, F1, P, B_LOC], xf_dt,
                                   name=f"a2aout{q}") for q in range(4)]
            ar_in = dram.tile([HW_W, B], f32)
            ag_out = dram.tile([NC_CORES, HW_W, B], f32)

            # ================= mp1 + L1 feature matmul =================
            def mp_layer(x_dram, elem, out_cb, tiles, gi_sb, nr_sb, of_sb,
                         dt=f32, order=None, elem_u=None, early_split=None,
                         hooks=None):
                """gather + scatter for one GCN layer; out_cb(t, agg_psum_ap).

                One dma_gather covers GSZ consecutive tiles of the processing
                order (their chunks are packed contiguously). With
                early_split (maps tile -> early-chunk count), each tile's
                early chunks gather from the first half of x_dram only, so
                those gathers depend on half the producer writes and can
                hoist. hooks[i] is emitted after tile position i (used to
                interleave collective issue into this engine queue's program
                order). One-hot scatter matrices
                S[c][e, n] = norm[e] * (dstoff[e] == n) are built on DVE."""
                if elem_u is None:
                    elem_u = elem
                seq = list(order) if order is not None else list(range(NTILES))
                for gi in range(0, NTILES, GSZ):
                    gts = seq[gi:gi + GSZ]
                    g_cs = [c for t in gts for c in tiles[t]]
                    gc0, gnch = g_cs[0], len(g_cs)
                    assert g_cs == list(range(gc0, gc0 + gnch)), \
                        "group chunks must be contiguous (pack order mismatch)"
                    msg = sb_msg.tile([P, gnch * elem], dt, tag="msg")
                    msg3 = msg[:].rearrange("p (c e) -> p c e", e=elem)
                    if early_split is None:
                        nidx = gnch * P
                        nc.gpsimd.dma_gather(
                            out_ap=msg3,
                            in_ap=x_dram[:, :],
                            idxs_ap=gi_sb[:, gc0 * 8:(gc0 + gnch) * 8],
                            num_idxs=nidx,
                            num_idxs_reg=nidx,
                            elem_size=elem,
                        )
                    else:
                        assert GSZ == 1
                        t0 = gts[0]
                        ne = early_split[t0]
                        if ne > 0:
                            nc.gpsimd.dma_gather(
                                out_ap=msg3[:, 0:ne, :],
                                in_ap=x_dram[0:N // 2, :],
                                idxs_ap=gi_sb[:, gc0 * 8:(gc0 + ne) * 8],
                                num_idxs=ne * P,
                                num_idxs_reg=ne * P,
                                elem_size=elem,
                            )
                        if ne < gnch:
                            nc.gpsimd.dma_gather(
                                out_ap=msg3[:, ne:gnch, :],
                                in_ap=x_dram[:, :],
                                idxs_ap=gi_sb[:, (gc0 + ne) * 8:(gc0 + gnch) * 8],
                                num_idxs=(gnch - ne) * P,
                                num_idxs_reg=(gnch - ne) * P,
                                elem_size=elem,
                            )
                    for t in gts:
                        cs = tiles[t]
                        nch = len(cs)
                        s_t = sb_s.tile([P, nch * P], dt, tag="sC")
                        for i, c in enumerate(cs):
                            # S[e, n] = (iota[n] == dstoff[e]) * norm[e]
                            nc.vector.tensor_scalar(
                                out=s_t[:, i * P:(i + 1) * P], in0=iota_f[:],
                                scalar1=of_sb[:, c:c + 1], scalar2=nr_sb[:, c:c + 1],
                                op0=OP.is_equal, op1=OP.mult,
                            )
                        agg = ps_agg.tile([P, elem_u], f32, tag="agg")
                        for i, c in enumerate(cs):
                            mo = c - gc0
                            nc.tensor.matmul(
                                agg[:],
                                lhsT=s_t[:, i * P:(i + 1) * P],
                                rhs=msg[:, mo * elem:mo * elem + elem_u],
                                start=(i == 0),
                                stop=(i == nch - 1),
                            )
                        out_cb(t, agg)
                    if hooks is not None:
                        for pos in range(gi, gi + GSZ):
                            if pos in hooks:
                                hooks[pos]()

            # ---- layer 1
            def l1_out(t, agg):
                # transpose agg [128n, 64] -> aggT [64, 128n]
                agg_sb = sb.tile([P, BF1], f32, tag="agg1sb")
                nc.vector.tensor_copy(agg_sb[:], agg[:])
                psT = ps_t.tile([BF1, P], f32, tag="work")
                nc.tensor.transpose(psT[:], agg_sb[:], ident[:])
                aggT = sb.tile([BF1 + 1, P], f32, tag="aggT1")
                nc.vector.tensor_copy(aggT[0:BF1, :], psT[:])
                nc.vector.memset(aggT[BF1:BF1 + 1, :], 1.0)
                # H1[t] = relu(aggT_aug.T @ w1bd_aug)  (last row carries b1)
                psH = ps_t.tile([P, B_LOC * F1], f32, tag="work")
                nc.tensor.matmul(psH[:], lhsT=aggT[:], rhs=w1bd_sb[:], start=True, stop=True)
                h1slice = h1_keep[:, t * BFH:(t + 1) * BFH]
                nc.scalar.activation(h1slice, psH[:], AF.Relu)
                nc.sync.dma_start(h1_dram[t * P:(t + 1) * P, :], h1slice)

            if phases[0]:
                # x rows padded to 128 bf16 cols (256B gather min); matmuls
                # only consume the first 64
                mp_layer(io["x_loc"], 2 * BF1, l1_out, tile_chunks1,
                         gidx_sb, nrm_sb, off_sb, dt=bf16, elem_u=BF1)

            # ---- layer 2 message passing -> a2a_in (feature-major [k, fin, n', b]
            # so the post-A2A W2-rhs reads are contiguous per partition)
            def l2_out(t, agg):
                # agg_sb = agg + dinv2[t] * H1[t]  (self-loop fold, one DVE
                # op); output already in the A2A's bf16 — the payload is bf16
                # anyway, and bf16 PE transposes run 2x faster than f32
                agg_sb = sb.tile([P, BFH], xf_dt, tag="agg2sb")
                nc.vector.scalar_tensor_tensor(
                    out=agg_sb[:], in0=h1_keep[:, t * BFH:(t + 1) * BFH],
                    scalar=dinv2_sb[:, t:t + 1], in1=agg[:],
                    op0=OP.mult, op1=OP.add,
                )
                a2a_sb = sb.tile([P, BFH], xf_dt, tag="a2asb")
                a2a_3d = a2a_sb[:].rearrange("f (n b) -> f n b", b=B_LOC)
                for b in range(B_LOC):
                    psT = ps_tx.tile([P, P], xf_dt, tag="workx")
                    nc.tensor.transpose(psT[:], agg_sb[:, b * F1:(b + 1) * F1], ident_x[:])
                    nc.vector.tensor_copy(a2a_3d[:, :, b], psT[:])
                k, q = t // 4, t % 4
                nc.sync.dma_start(a2a_in_q[q][k, :, :, :], a2a_sb[:])

            def a2a_chunk(q):
                if collectives:
                    nc.gpsimd.collective_compute(
                        "AllToAll",
                        mybir.AluOpType.bypass,
                        replica_groups=[list(range(NC_CORES))],
                        ins=[a2a_in_q[q][:].opt()],
                        outs=[a2a_out_q[q][:].opt()],
                    )
                else:
                    for s in range(NC_CORES):
                        nc.sync.dma_start(
                            a2a_out_q[q][s].rearrange("f n b -> f (n b)"),
                            a2a_in_q[q][s].rearrange("f n b -> f (n b)"),
                        )

            if phases[1]:
                # quarter-major order: all of quarter q's tiles finish
                # together, releasing A2A chunk q while mp2 continues on
                # quarter q+1. The A2A issue is interleaved into the Pool
                # queue ~5 gathers into the NEXT quarter: by then quarter q's
                # compute is done, so the collective's input wait does not
                # head-of-line-block the remaining gathers (the transfer
                # itself runs async on the CCOM rings).
                hooks = None
                if A2A_EARLY:
                    hooks = {12: lambda: a2a_chunk(0),
                             20: lambda: a2a_chunk(1),
                             28: lambda: a2a_chunk(2),
                             31: lambda: a2a_chunk(3)}
                mp_layer(h1_dram, BFH, l2_out, tile_chunks2,
                         gidx2_sb, nrm2_sb, off2_sb, dt=mp2_dt,
                         order=MP2_ORDER,
                         early_split=(n_early2 if SRCSPLIT else None),
                         hooks=hooks)

            # ====== per-quarter AllToAll chunk + W2 + head partials ======
            # Head contraction runs operand-swapped: the h2 slab [128k, 32sb]
            # is the stationary operand (cheap 32-col weight loads) and the
            # headW slab [128k, 76] streams. Four consecutive k-slabs occupy
            # the PE's four 32-wide column groups (tile_position) and run
            # concurrently; ps_hd[32j:32j+32, :] holds col-group j's partial.
            NB_H = 64
            SB_COLS = NB_H * B_LOC                       # 256 cols per src core
            ps_hd = ps_head.tile([P, HW_W], f32)
            nblocks = (NSLICE // NB_H) if phases[2] else 0
            n_slab = nblocks * NB_H * 2
            slab_ctr = 0
            if not phases[2]:
                nc.vector.memset(ps_hd[:], 0.0)
            elif TILEPOS:
                # col-group accumulation shares one bank; start=True clears
                # has_written BANK-wide, so zero the whole region once with a
                # dummy matmul and accumulate everything else with start=False
                zero_w = const.tile([P, P], f32)
                nc.vector.memset(zero_w[:], 0.0)
                nc.tensor.matmul(ps_hd[:], lhsT=zero_w[:],
                                 rhs=ident[:, 0:HW_W], start=True, stop=False,
                                 skip_group_check=True)

            for nb in range(nblocks):
                q, half = nb // 2, nb % 2
                if half == 0 and not (A2A_EARLY and phases[1]):
                    a2a_chunk(q)
                # stage rhs [128 fin, (s, n, b)]: 512B runs per partition
                rhs_sb = sb.tile([P, NC_CORES * SB_COLS], xf_dt, tag="w2rhs")
                if RHS_MERGE:
                    nc.sync.dma_start(
                        rhs_sb[:].rearrange("p (s n b) -> p s n b",
                                            s=NC_CORES, n=NB_H, b=B_LOC),
                        a2a_out_q[q][:, :, half * NB_H:(half + 1) * NB_H, :]
                        .rearrange("s f n b -> f s n b"),
                    )
                else:
                    for s in range(NC_CORES):
                        nc.sync.dma_start(
                            rhs_sb[:, s * SB_COLS:(s + 1) * SB_COLS],
                            a2a_out_q[q][s, :, half * NB_H:(half + 1) * NB_H, :],
                        )
                h2 = []
                for fh in range(2):
                    # h2 stored node-major (n, s, b) so the head lhsT slab
                    # [128, 32] for node i is a contiguous column slice
                    h2sb = sb.tile([P, NC_CORES * SB_COLS], hd_dt, tag="h2sb")
                    h2v = h2sb[:].rearrange("p (n s b) -> p n s b",
                                            n=NB_H, s=NC_CORES, b=B_LOC)
                    for qq in range(4):  # free split: 512-col matmuls
                        sl = slice(qq * 512, (qq + 1) * 512)
                        psW = ps_t.tile([P, 512], f32, tag="work")
                        nc.tensor.matmul(
                            psW[:], lhsT=w2_sb[:, fh * P:(fh + 1) * P],
                            rhs=rhs_sb[:, sl], start=True, stop=True,
                        )
                        # relu + per-partition bias b2[fh*128 + p]; psW's
                        # column order is (s, n, b) for s in {2qq, 2qq+1}
                        nc.scalar.activation(
                            h2v[:, :, 2 * qq:2 * qq + 2, :]
                            .rearrange("p n s b -> p s n b"),
                            psW[:].rearrange("p (s n b) -> p s n b",
                                             s=2, n=NB_H, b=B_LOC),
                            AF.Relu, bias=b2a[:, fh:fh + 1])
                    h2.append(h2sb)
                hw_sb = None
                for i in range(NB_H):
                    g = (nb * NB_H + i) // 16          # 16-node headW slab index
                    if nb < PRE_NB:
                        hw_view = hw_pre[:, g * 32 * HW_P:(g + 1) * 32 * HW_P]
                    else:
                        if i % 16 == 0:
                            hw_sb = sb_hw.tile([P, 32 * HW_P], hd_dt, tag="hwslab")
                            nc.sync.dma_start(
                                hw_sb[:],
                                io["headw_t"][:, g * 32 * HW_P:(g + 1) * 32 * HW_P])
                        hw_view = hw_sb[:]
                    for fh in range(2):
                        jj = 2 * (i % 16) + fh
                        j4 = slab_ctr % N_CG
                        nc.tensor.matmul(
                            ps_hd[32 * j4:32 * (j4 + 1), :],
                            lhsT=h2[fh][:, i * 32:(i + 1) * 32],
                            rhs=hw_view[:, jj * HW_P:(jj + 1) * HW_P],
                            start=(not TILEPOS) and slab_ctr == 0,
                            stop=(slab_ctr >= n_slab - N_CG),
                            tile_position=(0, 32 * j4) if TILEPOS else None,
                            skip_group_check=True,
                        )
                        slab_ctr += 1

            # ps_hd [N_CG*32b, 76] -> transpose -> [76, N_CG*32b] -> sum groups
            hd_sb = sb.tile([32 * N_CG, HW_W], f32, tag="hdsb")
            nc.vector.tensor_copy(hd_sb[:], ps_hd[0:32 * N_CG, :])
            psTr = ps_t.tile([HW_W, 32 * N_CG], f32, tag="work")
            nc.tensor.transpose(psTr[:], hd_sb[:],
                                ident[0:32 * N_CG, 0:32 * N_CG])
            tr_sb = sb.tile([HW_W, 32 * N_CG], f32, tag="trsb")
            nc.vector.tensor_copy(tr_sb[:], psTr[:])
            if N_CG == 4:
                hp0 = sb.tile([HW_W, 2 * B], f32, tag="hp0")
                nc.vector.scalar_tensor_tensor(
                    out=hp0[:, 0:B], in0=tr_sb[:, 0:B], scalar=1.0,
                    in1=tr_sb[:, B:2 * B], op0=OP.mult, op1=OP.add)
                nc.vector.scalar_tensor_tensor(
                    out=hp0[:, B:2 * B], in0=tr_sb[:, 2 * B:3 * B], scalar=1.0,
                    in1=tr_sb[:, 3 * B:4 * B], op0=OP.mult, op1=OP.add)
                part_sb = sb.tile([HW_W, B], f32, tag="part")
                nc.vector.scalar_tensor_tensor(
                    out=part_sb[:], in0=hp0[:, 0:B], scalar=1.0,
                    in1=hp0[:, B:2 * B], op0=OP.mult, op1=OP.add)
            else:
                part_sb = tr_sb
            nc.sync.dma_start(ar_in[:, :], part_sb[:, 0:B])

            # ============ AllGather partials + on-chip tree sum ============
            # (AllGather + 7 DVE adds beats AllReduce for a 9.7KB payload:
            # no reduce phase, ~half the collective latency)
            if collectives:
                nc.gpsimd.collective_compute(
                    "AllGather",
                    mybir.AluOpType.bypass,
                    replica_groups=[list(range(NC_CORES))],
                    ins=[ar_in[:].opt()],
                    outs=[ag_out[:].opt()],
                )
            else:
                for s in range(NC_CORES):
                    nc.sync.dma_start(ag_out[s, :, :], ar_in[:, :])
            red8_sb = sb.tile([HW_W, NC_CORES * B], f32, tag="red8")
            nc.sync.dma_start(
                red8_sb[:].rearrange("h (s b) -> h s b", s=NC_CORES, b=B),
                ag_out[:].rearrange("s h b -> h s b"),
            )
            for j in range(4):
                nc.vector.scalar_tensor_tensor(
                    out=red8_sb[:, j * B:(j + 1) * B],
                    in0=red8_sb[:, j * B:(j + 1) * B], scalar=1.0,
                    in1=red8_sb[:, (j + 4) * B:(j + 5) * B],
                    op0=OP.mult, op1=OP.add)
            for j in range(2):
                nc.vector.scalar_tensor_tensor(
                    out=red8_sb[:, j * B:(j + 1) * B],
                    in0=red8_sb[:, j * B:(j + 1) * B], scalar=1.0,
                    in1=red8_sb[:, (j + 2) * B:(j + 3) * B],
                    op0=OP.mult, op1=OP.add)
            red_sb = sb.tile([HW_W, B], f32, tag="red")
            nc.vector.scalar_tensor_tensor(
                out=red_sb[:], in0=red8_sb[:, 0:B], scalar=1.0,
                in1=red8_sb[:, B:2 * B], op0=OP.mult, op1=OP.add)

            # ================= final MLP + dueling combine =================
            adv_sb = sb.tile([12, B], f32, tag="adv")
            nc.scalar.activation(adv_sb[:], red_sb[64:76, :], AF.Relu, bias=advb_sb[:])
            # val path
            v1_sb = sb.tile([64, B], f32, tag="v1")
            nc.scalar.activation(v1_sb[:], red_sb[0:64, :], AF.Relu, bias=v1b_sb[:])
            psV = ps_t.tile([64, B], f32, tag="work")
            nc.tensor.matmul(psV[:], lhsT=v2w_sb[:], rhs=v1_sb[:], start=True, stop=True)
            v2_sb = sb.tile([64, B], f32, tag="v2")
            nc.scalar.activation(v2_sb[:], psV[:], AF.Relu, bias=v2b_sb[:])
            psV3 = ps_t.tile([1, B], f32, tag="work")
            nc.tensor.matmul(psV3[:], lhsT=v3w_sb[:], rhs=v2_sb[:], start=True, stop=True)
            val_sb = sb.tile([1, B], f32, tag="val")
            nc.vector.tensor_scalar_add(val_sb[:], psV3[:], v3b_sb[0:1, 0:1])
            # out = cmat.T @ adv + 1.T @ val
            psO = ps_t.tile([12, B], f32, tag="work")
            nc.tensor.matmul(psO[:], lhsT=cmat_sb[:], rhs=adv_sb[:], start=True, stop=False)
            nc.tensor.matmul(psO[:], lhsT=ones1[:, 0:12], rhs=val_sb[:], start=False, stop=True)
            out_sb = sb.tile([12, B], f32, tag="out")
            nc.vector.tensor_copy(out_sb[:], psO[:])
            nc.sync.dma_start(io["out"][:, :], out_sb[:])


# ---------------- driver ----------------

LAST_RESULTS = None

def _input_specs(shared, per_core):
    """name -> (shape, np dtype); per-core entries use per_core[0] shapes."""
    specs = {}
    for k, v in shared.items():
        specs[k] = v
    for k, v in per_core[0].items():
        specs[k] = v
    return specs


def kernel(**inputs) -> np.ndarray:
    import concourse.bacc as bacc
    import concourse.mybir as mybir
    import concourse.tile as tile
    from concourse import bass_utils

    shared, per_core, chunk_tile, nchunk = _prep_host(inputs)

    nc = bacc.Bacc("TRN2", target_bir_lowering=False, debug=False,
                   enable_asserts=False, num_devices=NC_CORES)

    io = {}
    specs = _input_specs(shared, per_core)
    for name, arr in specs.items():
        io[name] = nc.dram_tensor(
            name, list(arr.shape), mybir.dt.from_np(arr.dtype), kind="ExternalInput"
        ).ap()
    io["out"] = nc.dram_tensor(
        "out", [12, B], mybir.dt.float32, kind="ExternalOutput"
    ).ap()

    with tile.TileContext(nc) as tc:
        build_program(nc, tc, chunk_tile, nchunk, io)
    nc.compile()

    in_maps = []
    for j in range(NC_CORES):
        m = dict(shared)
        m.update(per_core[j])
        in_maps.append(m)

    res = bass_utils.run_bass_kernel_spmd(
        nc, in_maps, core_ids=list(range(NC_CORES)),
    )
    global LAST_RESULTS
    LAST_RESULTS = res
    out = res.results[0]["out"]                      # [12, 32]
    return out.T.reshape(B, 3, 4).copy().astype(np.float32)


if __name__ == "__main__":
    rng = np.random.default_rng(0)
    ei = rng.integers(0, N, (2, E)).astype(np.int64)
    demo = {
        "x": rng.standard_normal((B, N, F_IN), np.float32),
        "edge_index": ei,
        "edge_weight": rng.random(E, np.float32),
        "W1": rng.standard_normal((F_IN, F1), np.float32) / 4,
        "b1": np.zeros(F1, np.float32),
        "W2": rng.standard_normal((F1, F2), np.float32) / 11.3,
        "b2": np.zeros(F2, np.float32),
        "advW": rng.standard_normal((N * F2, 12), np.float32) / 1024,
        "advb": np.zeros(12, np.float32),
        "v1W": rng.standard_normal((N * F2, 64), np.float32) / 1024,
        "v1b": np.zeros(64, np.float32),
        "v2W": rng.standard_normal((64, 64), np.float32) / 8,
        "v2b": np.zeros(64, np.float32),
        "v3W": rng.standard_normal((64, 1), np.float32) / 8,
        "v3b": np.zeros(1, np.float32),
    }
    print(kernel(**demo).shape)



# revision 80
# speedup vs baseline: 1.3728x; 1.1212x over previous
"""Trainium2 Bass kernel for BHS_GCN: 2x GCNConv + dueling value/advantage heads.

Strategy (8 NeuronCores, single NEFF launch, bf16 compute / fp32 PSUM):
  - GCN phase batch-parallel: each core owns B_LOC=4 full graphs.
    Message passing = per-tile dma_gather of source-node rows (kept under the
    1024-descriptor SWDGE ring limit) + PE one-hot scatter-matmuls into PSUM
    (edges pre-sorted/packed by dst on host). Self-loop terms are NOT in the
    edge list: they are folded on DVE as agg += dinv2*H in the same op as the
    PSUM->SBUF copy (H1 kept in SBUF), cutting 20% of gather traffic.
  - mp1 gathers x as bf16 rows padded to 256B; layer matmuls and one-hot
    builds run at bf16 PE/DVE rates.
  - AllToAll (4 pipelined quarter-chunks, bf16) reshards the pre-W2
    aggregation to node-parallel: each core gets its 512-node slice for all
    32 batches, so each core streams only its 1/8 of advW/v1W (159MB bf16
    machine-wide, read once, unpadded 76-col tiles; first PRE_NB blocks
    prefetched into SBUF).
  - Head contraction is operand-swapped: the h2 slab [128k, 32sb] is the
    stationary operand (cheap weight loads) and headW streams; four k-slabs
    run concurrently in the PE's four 32-col groups (tile_position), with a
    single bank-wide dummy-clear then start=False accumulation (start=True
    clears has_written bank-wide).
  - AllGather of [76,32] partial head sums + on-chip tree-sum (cheaper than
    AllReduce for a 9.7KB payload); the tiny val-MLP and dueling combine run
    redundantly on every core; host takes core 0's output.
"""

import sys

sys.path.insert(0, "/opt/trn_rl_repo")

import os

import numpy as np
import ml_dtypes

# Precision mode: "f32" (exact), "bf16" (everything big in bf16), or a
# comma-set of {mp2,xfer,head}: mp2 = H1/messages/one-hots; xfer = A2A
# payload + W2; head = H2 + head weights. Accumulation is always fp32 PSUM.
PRECISION = os.environ.get("GCN_PREC", "bf16")
BF16 = np.dtype(ml_dtypes.bfloat16)


def _prec_groups():
    if PRECISION == "f32":
        return set()
    if PRECISION == "bf16":
        return {"mp2", "xfer", "head", "l1"}
    return set(PRECISION.split(","))


PREC_G = _prec_groups()

# ---------------- problem constants (hardcoded per contract) ----------------
B, N, F_IN, E = 32, 4096, 16, 16384
NC_CORES = 8
B_LOC = B // NC_CORES            # 4
NSLICE = N // NC_CORES           # 512 nodes per core for head phase
F1, F2 = 128, 256
P = 128
NTILES = N // P                  # 32 node tiles
BF1 = B_LOC * F_IN               # 64   (mp1 row width)
BFH = B_LOC * F1                 # 512  (H1 row width = mp2 gather width)
KTOT = NSLICE * F2               # 131072 contraction rows per core
KT = KTOT // P                   # 1024 K-tiles for head matmul
HW_W = 12 + 64                   # 76 head outputs (adv | v1)
# unpadded head-weight tiles: FWL would need 128 cols but the 68% extra
# HBM traffic costs more than the slower ldweights saves (DMA-bound kernel)
HW_P = HW_W
NT_HEAD = 16                     # nodes per W2/head block
PRE_NB = int(os.environ.get("GCN_PRE", "1"))   # head-weight nb-blocks in SBUF
# One dma_gather per node tile: a gather's descriptors must fit the 1024-slot
# SWDGE ring (single doorbell fires only after full emission — a gather
# bigger than the ring deadlocks on HW; the interpreter does not model this)
GSZ = int(os.environ.get("GCN_GSZ", "1"))
MSGBUF = int(os.environ.get("GCN_MSGBUF", "5"))  # msg/S pool depth
TILEPOS = os.environ.get("GCN_TILEPOS", "1") != "0"  # 4-way PE col tiling
N_CG = 4 if TILEPOS else 1       # PE col groups used by the head contraction
RHS_MERGE = os.environ.get("GCN_RHSMERGE", "1") != "0"  # single rhs-stage DMA
SRCSPLIT = os.environ.get("GCN_SRCSPLIT", "1") != "0"  # mp2 early/late gathers
# Interleaving A2A issue among the gathers REGRESSES on HW: the collective
# instruction occupies the GPSIMD queue for its whole transfer, stalling the
# gathers queued behind it. Keep collectives after all gathers (default off).
A2A_EARLY = os.environ.get("GCN_A2AEARLY", "0") != "0"
# mp2 processes tiles quarter-major so A2A chunk q releases after 8 tiles
MP2_ORDER = [4 * k + q for q in range(4) for k in range(8)]


def _pack_sorted(src_a, dst_a, nrm_a, tile_order=None, src_split=None):
    """Sort edges by dst, pack into 128-edge chunks such that every chunk's
    dsts fall in one 128-node tile. Tiles are packed in `tile_order` so that
    the chunks of GSZ consecutive tiles in that order are contiguous (one
    dma_gather per tile group). With src_split, each tile's chunks are packed
    early-src-first (src < src_split) so the early gather can read a
    DRAM slice that is ready before the whole H1 is written; returns the
    per-tile early-chunk count in that case."""
    order = np.argsort(dst_a, kind="stable")
    src_a, dst_a, nrm_a = src_a[order], dst_a[order], nrm_a[order]

    src_pk, nrm_pk, off_pk = [], [], []
    chunk_tile = []
    n_early = {}
    for t in (tile_order if tile_order is not None else range(NTILES)):
        sel = (dst_a >= t * P) & (dst_a < (t + 1) * P)
        s, d, w = src_a[sel], dst_a[sel], nrm_a[sel]
        if src_split is not None:
            parts = []
            for half, hsel in ((0, s < src_split), (1, s >= src_split)):
                sh, dh, wh = s[hsel], d[hsel], w[hsel]
                cnt = len(sh)
                nch_h = (cnt + P - 1) // P
                pad = nch_h * P - cnt
                parts.append((
                    np.concatenate([sh, np.zeros(pad, np.int64)]),
                    np.concatenate([wh, np.zeros(pad, np.float32)]),
                    np.concatenate([dh - t * P, np.zeros(pad, np.int64)]),
                    nch_h))
            if parts[0][3] + parts[1][3] == 0:
                parts[0] = (np.zeros(P, np.int64), np.zeros(P, np.float32),
                            np.zeros(P, np.int64), 1)
            n_early[t] = parts[0][3]
            src_pk.extend([parts[0][0], parts[1][0]])
            nrm_pk.extend([parts[0][1], parts[1][1]])
            off_pk.extend([parts[0][2], parts[1][2]])
            chunk_tile.extend([t] * (parts[0][3] + parts[1][3]))
            continue
        cnt = len(s)
        nch = max(1, (cnt + P - 1) // P)
        pad = nch * P - cnt
        src_pk.append(np.concatenate([s, np.zeros(pad, np.int64)]))
        nrm_pk.append(np.concatenate([w, np.zeros(pad, np.float32)]))
        off_pk.append(np.concatenate([d - t * P, np.zeros(pad, np.int64)]))
        chunk_tile.extend([t] * nch)

    src_pk = np.concatenate(src_pk)
    nrm_pk = np.concatenate(nrm_pk)
    off_pk = np.concatenate(off_pk)
    e_pad = len(src_pk)
    nchunk = e_pad // P
    assert nchunk == len(chunk_tile)

    # dma_gather index table: logical idx i lives at [i % 16, i // 16]
    gidx = np.zeros((P, e_pad // 16), np.int16)
    for p16 in range(16):
        gidx[p16, :] = src_pk[p16::16].astype(np.int16)
    gidx = np.tile(gidx[:16], (8, 1))  # replicate over all 128 partitions

    # per-chunk column tables: [p, c] = value of edge c*128+p
    nrm_t = nrm_pk.reshape(nchunk, P).T.copy()          # [128, nchunk] f32
    off_t = off_pk.reshape(nchunk, P).T.astype(np.float32).copy()
    return gidx, nrm_t, off_t, chunk_tile, nchunk, n_early


def _pack_edges(edge_index, edge_weight):
    """Two packings: mp1 includes self-loop edges (x stays in DRAM only);
    mp2 excludes them — the self-loop term is folded on DVE from the
    SBUF-resident H1 (saves 20% of gather traffic and chunk matmuls).
    Also returns dinv2 [128, NTILES]: 1/deg for node t*128+p."""
    src = np.asarray(edge_index[0], np.int64)
    dst = np.asarray(edge_index[1], np.int64)
    ew = np.asarray(edge_weight, np.float32)

    deg = np.zeros(N, np.float32)
    np.add.at(deg, dst, ew)
    deg += 1.0
    dinv = (1.0 / np.sqrt(deg)).astype(np.float32)
    norm = ew * dinv[src] * dinv[dst]

    # mp1: edges + self loops (src=dst=n, weight 1/deg[n])
    src_a = np.concatenate([src, np.arange(N, dtype=np.int64)])
    dst_a = np.concatenate([dst, np.arange(N, dtype=np.int64)])
    nrm_a = np.concatenate([norm, dinv * dinv]).astype(np.float32)
    t1 = _pack_sorted(src_a, dst_a, nrm_a)
    # mp2: edges only, packed in mp2's quarter-major processing order; each
    # tile's chunks are early-src-first so the early gather only depends on
    # the first half of H1
    t2 = _pack_sorted(src, dst, norm.astype(np.float32), tile_order=MP2_ORDER,
                      src_split=(N // 2 if SRCSPLIT else None))

    dinv2_t = (dinv * dinv).reshape(NTILES, P).T.copy()  # [128, NTILES]
    return t1, t2, dinv2_t


def _prep_host(inputs):
    """All host-side numpy preprocessing: edge packing, weight layout, batch shard."""
    x = np.asarray(inputs["x"], np.float32)
    (gidx, nrm_t, off_t, chunk_tile, nchunk, _), \
        (gidx2, nrm_t2, off_t2, chunk_tile2, nchunk2, n_early2), dinv2_t = \
        _pack_edges(inputs["edge_index"], inputs["edge_weight"])

    W1 = np.asarray(inputs["W1"], np.float32)      # [16,128]
    b1 = np.asarray(inputs["b1"], np.float32)      # [128]
    W2 = np.asarray(inputs["W2"], np.float32)      # [128,256]
    b2 = np.asarray(inputs["b2"], np.float32)      # [256]
    advW = np.asarray(inputs["advW"], np.float32)  # [N*256, 12]
    advb = np.asarray(inputs["advb"], np.float32)
    v1W = np.asarray(inputs["v1W"], np.float32)    # [N*256, 64]
    v1b = np.asarray(inputs["v1b"], np.float32)
    v2W = np.asarray(inputs["v2W"], np.float32)
    v2b = np.asarray(inputs["v2b"], np.float32)
    v3W = np.asarray(inputs["v3W"], np.float32)
    v3b = np.asarray(inputs["v3b"], np.float32)

    # W1 block-diagonal over the 4 local batches, plus a bias row driven by
    # a constant-1 row appended to aggT on device: [65, 512]
    w1bd = np.zeros((BF1 + 1, B_LOC * F1), np.float32)
    for b in range(B_LOC):
        w1bd[b * F_IN:(b + 1) * F_IN, b * F1:(b + 1) * F1] = W1
    w1bd[BF1, :] = np.tile(b1, B_LOC)

    # dueling combine matrix (adv part): out = C.T @ adv + val
    C = np.zeros((12, 12), np.float32)
    for h in range(3):
        for a in range(4):
            i = h * 4 + a
            C[i, i] += 1.0
            for a2 in range(4):
                C[h * 4 + a2, i] -= 0.25

    shared = {
        "gidx": gidx,
        "nrm_t": nrm_t.copy(),
        "off_t": off_t.copy(),
        "gidx2": gidx2,
        "nrm_t2": nrm_t2.copy(),
        "off_t2": off_t2.copy(),
        "dinv2_t": dinv2_t.copy(),
        "w1bd": (w1bd.astype(BF16) if "l1" in PREC_G else w1bd).copy(),
        "w2": (W2.astype(BF16) if "xfer" in PREC_G else W2).copy(),
        "b2c": b2[:, None].copy(),                  # [256,1]
        "advb_c": advb[:, None].copy(),             # [12,1]
        "v1b_c": v1b[:, None].copy(),               # [64,1]
        "v2w": v2W.copy(),                          # [64,64]
        "v2b_c": v2b[:, None].copy(),               # [64,1]
        "v3w": v3W.copy(),                          # [64,1]
        "v3b_c": v3b[None, :].copy(),               # [1,1]
        "cmat": C,
    }

    per_core = []
    for j in range(NC_CORES):
        # x batch-shard, node-major rows [N, b, f] -> [N, 64], bf16 padded to
        # 128 cols (gather elem_size_bytes must be a multiple of 256)
        x_nb = x[j * B_LOC:(j + 1) * B_LOC].transpose(1, 0, 2).reshape(N, BF1)
        x_loc = np.zeros((N, 2 * BF1), BF16)
        x_loc[:, :BF1] = x_nb.astype(BF16)
        # head weights: rows for this core's node slice, pre-tiled to
        # [128, KT*76]: col block j holds lhsT K-tile j = rows [128j,128j+128)
        r0 = j * KTOT
        aw = advW[r0:r0 + KTOT].reshape(KT, P, 12)
        vw = v1W[r0:r0 + KTOT].reshape(KT, P, 64)
        # v1 first (partitions 0:64), adv second (64:76): partition slices
        # must start at multiples of 32 on-device.
        hw = np.concatenate([vw, aw], axis=2)  # [KT, 128, 76]
        hw_t = hw.transpose(1, 0, 2).reshape(P, KT * HW_P)
        hw_t = (hw_t.astype(BF16) if "head" in PREC_G else hw_t).copy()
        per_core.append({"x_loc": x_loc, "headw_t": hw_t})

    return shared, per_core, (chunk_tile, chunk_tile2, n_early2), (nchunk, nchunk2)


# ---------------- device program ----------------

def build_program(nc, tc, chunk_tile, nchunk, io, collectives=True, phases=(1,1,1), repeat=1):
    """Emit the Tile program. io: dict of name -> DRAM AP."""
    import concourse.bass as bass
    import concourse.mybir as mybir
    import concourse.tile as tile
    from concourse.masks import make_identity

    f32 = mybir.dt.float32
    f32r = mybir.dt.float32r
    bf16 = mybir.dt.bfloat16
    mp2_dt = bf16 if "mp2" in PREC_G else f32
    xf_dt = bf16 if "xfer" in PREC_G else f32
    hd_dt = bf16 if "head" in PREC_G else f32
    l1_dt = bf16 if "l1" in PREC_G else f32
    i16 = mybir.dt.int16
    i32 = mybir.dt.int32
    AF = mybir.ActivationFunctionType
    OP = mybir.AluOpType

    chunk_tile1, chunk_tile2, n_early2 = chunk_tile
    nchunk1, nchunk2 = nchunk
    # chunks belonging to each node tile (contiguous ranges), per layer
    tile_chunks1 = [[] for _ in range(NTILES)]
    for c, t in enumerate(chunk_tile1):
        tile_chunks1[t].append(c)
    tile_chunks2 = [[] for _ in range(NTILES)]
    for c, t in enumerate(chunk_tile2):
        tile_chunks2[t].append(c)

    from contextlib import ExitStack
    with ExitStack() as ctx:
        const = ctx.enter_context(tc.tile_pool(name="const", bufs=1))
        sb = ctx.enter_context(tc.tile_pool(name="sb", bufs=3))
        sb_msg = ctx.enter_context(tc.tile_pool(name="msg", bufs=MSGBUF))
        sb_s = ctx.enter_context(tc.tile_pool(name="sbs", bufs=MSGBUF))
        sb_hw = ctx.enter_context(tc.tile_pool(name="sbhw", bufs=3))
        ps_agg = ctx.enter_context(tc.tile_pool(name="ps_agg", bufs=2, space="PSUM"))
        ps_t = ctx.enter_context(tc.tile_pool(name="ps_t", bufs=3, space="PSUM"))
        ps_tx = ctx.enter_context(tc.tile_pool(name="ps_tx", bufs=2, space="PSUM"))
        ps_head = ctx.enter_context(tc.tile_pool(name="ps_head", bufs=1, space="PSUM"))
        dram = ctx.enter_context(tc.tile_pool(name="dram", bufs=1, space="DRAM"))
        for _rep in range(repeat):
            # ---- constants into SBUF
            ident = const.tile([P, P], f32)
            make_identity(nc, ident[:])
            ident_x = const.tile([P, P], xf_dt)
            nc.vector.tensor_copy(ident_x[:], ident[:])
            iota_i = const.tile([P, P], i32)
            nc.gpsimd.iota(iota_i[:], pattern=[[1, P]], base=0, channel_multiplier=0)
            iota_f = const.tile([P, P], f32)
            nc.vector.tensor_copy(iota_f[:], iota_i[:])
            ones1 = const.tile([1, P], f32)
            nc.vector.memset(ones1[:], 1.0)

            gidx_sb = const.tile([P, nchunk1 * 8], i16)
            nc.sync.dma_start(gidx_sb[:], io["gidx"][:, :])
            nrm_sb = const.tile([P, nchunk1], f32)
            nc.sync.dma_start(nrm_sb[:], io["nrm_t"][:, :])
            off_sb = const.tile([P, nchunk1], f32)
            nc.sync.dma_start(off_sb[:], io["off_t"][:, :])
            gidx2_sb = const.tile([P, nchunk2 * 8], i16)
            nc.sync.dma_start(gidx2_sb[:], io["gidx2"][:, :])
            nrm2_sb = const.tile([P, nchunk2], f32)
            nc.sync.dma_start(nrm2_sb[:], io["nrm_t2"][:, :])
            off2_sb = const.tile([P, nchunk2], f32)
            nc.sync.dma_start(off2_sb[:], io["off_t2"][:, :])
            dinv2_sb = const.tile([P, NTILES], f32)
            nc.sync.dma_start(dinv2_sb[:], io["dinv2_t"][:, :])
            # SBUF-resident H1 (written by mp1, read by mp2's self-loop fold)
            h1_keep = const.tile([P, NTILES * BFH], mp2_dt)
            # head-weight prefetch: first PRE_NB nb-blocks' slabs (128*HW_P
            # cols each), loaded at kernel start to use DMA-idle time during
            # the PE-heavy GCN phase
            pre_cols = PRE_NB * 128 * HW_P
            hw_pre = const.tile([P, pre_cols], hd_dt)
            nc.sync.dma_start(hw_pre[:], io["headw_t"][:, 0:pre_cols])

            w1bd_sb = const.tile([BF1 + 1, B_LOC * F1], l1_dt)
            nc.sync.dma_start(w1bd_sb[:], io["w1bd"][:, :])
            w2_sb = const.tile([P, F2], xf_dt)
            nc.sync.dma_start(w2_sb[:], io["w2"][:, :])
            # b2 [256,1] -> two [128,1] sbuf column stacks
            b2a = const.tile([P, 2], f32)
            nc.sync.dma_start(b2a[:, 0:1], io["b2c"][0:P, :])
            nc.sync.dma_start(b2a[:, 1:2], io["b2c"][P:F2, :])
            advb_sb = const.tile([12, 1], f32)
            nc.sync.dma_start(advb_sb[:], io["advb_c"][:, :])
            v1b_sb = const.tile([64, 1], f32)
            nc.sync.dma_start(v1b_sb[:], io["v1b_c"][:, :])
            v2w_sb = const.tile([64, 64], f32)
            nc.sync.dma_start(v2w_sb[:], io["v2w"][:, :])
            v2b_sb = const.tile([64, 1], f32)
            nc.sync.dma_start(v2b_sb[:], io["v2b_c"][:, :])
            v3w_sb = const.tile([64, 1], f32)
            nc.sync.dma_start(v3w_sb[:], io["v3w"][:, :])
            v3b_sb = const.tile([1, 1], f32)
            nc.sync.dma_start(v3b_sb[:], io["v3b_c"][:, :])
            cmat_sb = const.tile([12, 12], f32)
            nc.sync.dma_start(cmat_sb[:], io["cmat"][:, :])

            # scratch DRAM
            h1_dram = dram.tile([N, BFH], mp2_dt)        # node-major H1
            # agg2 feature-major, one buffer pair per node-quarter so the
            # AllToAll pipelines with mp2 and the head phase:
            # a2a_*_q[q][k, fin, n128, b]
            a2a_in_q = [dram.tile([NC_CORES, F1, P, B_LOC], xf_dt,
                                  name=f"a2ain{q}") for q in range(4)]
            a2a_out_q = [dram.tile([NC_CORES, F1, P, B_LOC], xf_dt,
                                   name=f"a2aout{q}") for q in range(4)]
            ar_in = dram.tile([HW_W, B], f32)
            ag_out = dram.tile([NC_CORES, HW_W, B], f32)

            # ================= mp1 + L1 feature matmul =================
            def mp_layer(x_dram, elem, out_cb, tiles, gi_sb, nr_sb, of_sb,
                         dt=f32, order=None, elem_u=None, early_split=None,
                         hooks=None, swap_out=False):
                """gather + scatter for one GCN layer; out_cb(t, agg_psum_ap).

                One dma_gather covers GSZ consecutive tiles of the processing
                order (their chunks are packed contiguously). With
                early_split (maps tile -> early-chunk count), each tile's
                early chunks gather from the first half of x_dram only, so
                those gathers depend on half the producer writes and can
                hoist. hooks[i] is emitted after tile position i (used to
                interleave collective issue into this engine queue's program
                order). One-hot scatter matrices
                S[c][e, n] = norm[e] * (dstoff[e] == n) are built on DVE."""
                if elem_u is None:
                    elem_u = elem
                seq = list(order) if order is not None else list(range(NTILES))
                for gi in range(0, NTILES, GSZ):
                    gts = seq[gi:gi + GSZ]
                    g_cs = [c for t in gts for c in tiles[t]]
                    gc0, gnch = g_cs[0], len(g_cs)
                    assert g_cs == list(range(gc0, gc0 + gnch)), \
                        "group chunks must be contiguous (pack order mismatch)"
                    msg = sb_msg.tile([P, gnch * elem], dt, tag="msg")
                    msg3 = msg[:].rearrange("p (c e) -> p c e", e=elem)
                    if early_split is None:
                        nidx = gnch * P
                        nc.gpsimd.dma_gather(
                            out_ap=msg3,
                            in_ap=x_dram[:, :],
                            idxs_ap=gi_sb[:, gc0 * 8:(gc0 + gnch) * 8],
                            num_idxs=nidx,
                            num_idxs_reg=nidx,
                            elem_size=elem,
                        )
                    else:
                        assert GSZ == 1
                        t0 = gts[0]
                        ne = early_split[t0]
                        if ne > 0:
                            nc.gpsimd.dma_gather(
                                out_ap=msg3[:, 0:ne, :],
                                in_ap=x_dram[0:N // 2, :],
                                idxs_ap=gi_sb[:, gc0 * 8:(gc0 + ne) * 8],
                                num_idxs=ne * P,
                                num_idxs_reg=ne * P,
                                elem_size=elem,
                            )
                        if ne < gnch:
                            nc.gpsimd.dma_gather(
                                out_ap=msg3[:, ne:gnch, :],
                                in_ap=x_dram[:, :],
                                idxs_ap=gi_sb[:, (gc0 + ne) * 8:(gc0 + gnch) * 8],
                                num_idxs=(gnch - ne) * P,
                                num_idxs_reg=(gnch - ne) * P,
                                elem_size=elem,
                            )
                    for t in gts:
                        cs = tiles[t]
                        nch = len(cs)
                        s_t = sb_s.tile([P, nch * P], dt, tag="sC")
                        for i, c in enumerate(cs):
                            # S[e, n] = (iota[n] == dstoff[e]) * norm[e]
                            nc.vector.tensor_scalar(
                                out=s_t[:, i * P:(i + 1) * P], in0=iota_f[:],
                                scalar1=of_sb[:, c:c + 1], scalar2=nr_sb[:, c:c + 1],
                                op0=OP.is_equal, op1=OP.mult,
                            )
                        if swap_out:
                            # transposed aggregate [elem_u, 128n] directly:
                            # lhsT = msg chunk (stationary), rhs = S (moving)
                            agg = ps_agg.tile([elem_u, P], f32, tag="agg")
                            for i, c in enumerate(cs):
                                mo = c - gc0
                                nc.tensor.matmul(
                                    agg[:],
                                    lhsT=msg[:, mo * elem:mo * elem + elem_u],
                                    rhs=s_t[:, i * P:(i + 1) * P],
                                    start=(i == 0),
                                    stop=(i == nch - 1),
                                )
                        else:
                            agg = ps_agg.tile([P, elem_u], f32, tag="agg")
                            for i, c in enumerate(cs):
                                mo = c - gc0
                                nc.tensor.matmul(
                                    agg[:],
                                    lhsT=s_t[:, i * P:(i + 1) * P],
                                    rhs=msg[:, mo * elem:mo * elem + elem_u],
                                    start=(i == 0),
                                    stop=(i == nch - 1),
                                )
                        out_cb(t, agg)
                    if hooks is not None:
                        for pos in range(gi, gi + GSZ):
                            if pos in hooks:
                                hooks[pos]()

            # ---- layer 1 (scatter matmul emits aggT [64, 128n] directly —
            # no PE transpose needed; last aggT row carries a constant 1 that
            # drives the b1 bias row of w1bd)
            def l1_out(t, aggT_ps):
                aggT = sb.tile([BF1 + 1, P], l1_dt, tag="aggT1")
                nc.vector.tensor_copy(aggT[0:BF1, :], aggT_ps[:])
                nc.vector.memset(aggT[BF1:BF1 + 1, :], 1.0)
                # H1[t] = relu(aggT_aug.T @ w1bd_aug)
                psH = ps_t.tile([P, B_LOC * F1], f32, tag="work")
                nc.tensor.matmul(psH[:], lhsT=aggT[:], rhs=w1bd_sb[:], start=True, stop=True)
                h1slice = h1_keep[:, t * BFH:(t + 1) * BFH]
                nc.scalar.activation(h1slice, psH[:], AF.Relu)
                nc.sync.dma_start(h1_dram[t * P:(t + 1) * P, :], h1slice)

            if phases[0]:
                # x rows padded to 128 bf16 cols (256B gather min); matmuls
                # only consume the first 64
                mp_layer(io["x_loc"], 2 * BF1, l1_out, tile_chunks1,
                         gidx_sb, nrm_sb, off_sb, dt=bf16, elem_u=BF1,
                         swap_out=True)

            # ---- layer 2 message passing -> a2a_in (feature-major [k, fin, n', b]
            # so the post-A2A W2-rhs reads are contiguous per partition)
            def l2_out(t, agg):
                # agg_sb = agg + dinv2[t] * H1[t]  (self-loop fold, one DVE
                # op); output already in the A2A's bf16 — the payload is bf16
                # anyway, and bf16 PE transposes run 2x faster than f32
                agg_sb = sb.tile([P, BFH], xf_dt, tag="agg2sb")
                nc.vector.scalar_tensor_tensor(
                    out=agg_sb[:], in0=h1_keep[:, t * BFH:(t + 1) * BFH],
                    scalar=dinv2_sb[:, t:t + 1], in1=agg[:],
                    op0=OP.mult, op1=OP.add,
                )
                a2a_sb = sb.tile([P, BFH], xf_dt, tag="a2asb")
                a2a_3d = a2a_sb[:].rearrange("f (n b) -> f n b", b=B_LOC)
                for b in range(B_LOC):
                    psT = ps_tx.tile([P, P], xf_dt, tag="workx")
                    nc.tensor.transpose(psT[:], agg_sb[:, b * F1:(b + 1) * F1], ident_x[:])
                    nc.vector.tensor_copy(a2a_3d[:, :, b], psT[:])
                k, q = t // 4, t % 4
                nc.sync.dma_start(a2a_in_q[q][k, :, :, :], a2a_sb[:])

            def a2a_chunk(q):
                if collectives:
                    nc.gpsimd.collective_compute(
                        "AllToAll",
                        mybir.AluOpType.bypass,
                        replica_groups=[list(range(NC_CORES))],
                        ins=[a2a_in_q[q][:].opt()],
                        outs=[a2a_out_q[q][:].opt()],
                    )
                else:
                    for s in range(NC_CORES):
                        nc.sync.dma_start(
                            a2a_out_q[q][s].rearrange("f n b -> f (n b)"),
                            a2a_in_q[q][s].rearrange("f n b -> f (n b)"),
                        )

            if phases[1]:
                # quarter-major order: all of quarter q's tiles finish
                # together, releasing A2A chunk q while mp2 continues on
                # quarter q+1. The A2A issue is interleaved into the Pool
                # queue ~5 gathers into the NEXT quarter: by then quarter q's
                # compute is done, so the collective's input wait does not
                # head-of-line-block the remaining gathers (the transfer
                # itself runs async on the CCOM rings).
                hooks = None
                if A2A_EARLY:
                    hooks = {12: lambda: a2a_chunk(0),
                             20: lambda: a2a_chunk(1),
                             28: lambda: a2a_chunk(2),
                             31: lambda: a2a_chunk(3)}
                mp_layer(h1_dram, BFH, l2_out, tile_chunks2,
                         gidx2_sb, nrm2_sb, off2_sb, dt=mp2_dt,
                         order=MP2_ORDER,
                         early_split=(n_early2 if SRCSPLIT else None),
                         hooks=hooks)

            # ====== per-quarter AllToAll chunk + W2 + head partials ======
            # Head contraction runs operand-swapped: the h2 slab [128k, 32sb]
            # is the stationary operand (cheap 32-col weight loads) and the
            # headW slab [128k, 76] streams. Four consecutive k-slabs occupy
            # the PE's four 32-wide column groups (tile_position) and run
            # concurrently; ps_hd[32j:32j+32, :] holds col-group j's partial.
            NB_H = 64
            SB_COLS = NB_H * B_LOC                       # 256 cols per src core
            ps_hd = ps_head.tile([P, HW_W], f32)
            nblocks = (NSLICE // NB_H) if phases[2] else 0
            n_slab = nblocks * NB_H * 2
            slab_ctr = 0
            if not phases[2]:
                nc.vector.memset(ps_hd[:], 0.0)
            elif TILEPOS:
                # col-group accumulation shares one bank; start=True clears
                # has_written BANK-wide, so zero the whole region once with a
                # dummy matmul and accumulate everything else with start=False
                zero_w = const.tile([P, P], f32)
                nc.vector.memset(zero_w[:], 0.0)
                nc.tensor.matmul(ps_hd[:], lhsT=zero_w[:],
                                 rhs=ident[:, 0:HW_W], start=True, stop=False,
                                 skip_group_check=True)

            for nb in range(nblocks):
                q, half = nb // 2, nb % 2
                if half == 0 and not (A2A_EARLY and phases[1]):
                    a2a_chunk(q)
                # stage rhs [128 fin, (s, n, b)]: 512B runs per partition
                rhs_sb = sb.tile([P, NC_CORES * SB_COLS], xf_dt, tag="w2rhs")
                if RHS_MERGE:
                    nc.sync.dma_start(
                        rhs_sb[:].rearrange("p (s n b) -> p s n b",
                                            s=NC_CORES, n=NB_H, b=B_LOC),
                        a2a_out_q[q][:, :, half * NB_H:(half + 1) * NB_H, :]
                        .rearrange("s f n b -> f s n b"),
                    )
                else:
                    for s in range(NC_CORES):
                        nc.sync.dma_start(
                            rhs_sb[:, s * SB_COLS:(s + 1) * SB_COLS],
                            a2a_out_q[q][s, :, half * NB_H:(half + 1) * NB_H, :],
                        )
                h2 = []
                for fh in range(2):
                    # h2 stored node-major (n, s, b) so the head lhsT slab
                    # [128, 32] for node i is a contiguous column slice
                    h2sb = sb.tile([P, NC_CORES * SB_COLS], hd_dt, tag="h2sb")
                    h2v = h2sb[:].rearrange("p (n s b) -> p n s b",
                                            n=NB_H, s=NC_CORES, b=B_LOC)
                    for qq in range(4):  # free split: 512-col matmuls
                        sl = slice(qq * 512, (qq + 1) * 512)
                        psW = ps_t.tile([P, 512], f32, tag="work")
                        nc.tensor.matmul(
                            psW[:], lhsT=w2_sb[:, fh * P:(fh + 1) * P],
                            rhs=rhs_sb[:, sl], start=True, stop=True,
                        )
                        # relu + per-partition bias b2[fh*128 + p]; psW's
                        # column order is (s, n, b) for s in {2qq, 2qq+1}
                        nc.scalar.activation(
                            h2v[:, :, 2 * qq:2 * qq + 2, :]
                            .rearrange("p n s b -> p s n b"),
                            psW[:].rearrange("p (s n b) -> p s n b",
                                             s=2, n=NB_H, b=B_LOC),
                            AF.Relu, bias=b2a[:, fh:fh + 1])
                    h2.append(h2sb)
                hw_sb = None
                for i in range(NB_H):
                    g = (nb * NB_H + i) // 16          # 16-node headW slab index
                    if nb < PRE_NB:
                        hw_view = hw_pre[:, g * 32 * HW_P:(g + 1) * 32 * HW_P]
                    else:
                        if i % 16 == 0:
                            hw_sb = sb_hw.tile([P, 32 * HW_P], hd_dt, tag="hwslab")
                            nc.sync.dma_start(
                                hw_sb[:],
                                io["headw_t"][:, g * 32 * HW_P:(g + 1) * 32 * HW_P])
                        hw_view = hw_sb[:]
                    for fh in range(2):
                        jj = 2 * (i % 16) + fh
                        j4 = slab_ctr % N_CG
                        nc.tensor.matmul(
                            ps_hd[32 * j4:32 * (j4 + 1), :],
                            lhsT=h2[fh][:, i * 32:(i + 1) * 32],
                            rhs=hw_view[:, jj * HW_P:(jj + 1) * HW_P],
                            start=(not TILEPOS) and slab_ctr == 0,
                            stop=(slab_ctr >= n_slab - N_CG),
                            tile_position=(0, 32 * j4) if TILEPOS else None,
                            skip_group_check=True,
                        )
                        slab_ctr += 1

            # ps_hd [N_CG*32b, 76] -> transpose -> [76, N_CG*32b] -> sum groups
            hd_sb = sb.tile([32 * N_CG, HW_W], f32, tag="hdsb")
            nc.vector.tensor_copy(hd_sb[:], ps_hd[0:32 * N_CG, :])
            psTr = ps_t.tile([HW_W, 32 * N_CG], f32, tag="work")
            nc.tensor.transpose(psTr[:], hd_sb[:],
                                ident[0:32 * N_CG, 0:32 * N_CG])
            tr_sb = sb.tile([HW_W, 32 * N_CG], f32, tag="trsb")
            nc.vector.tensor_copy(tr_sb[:], psTr[:])
            if N_CG == 4:
                hp0 = sb.tile([HW_W, 2 * B], f32, tag="hp0")
                nc.vector.scalar_tensor_tensor(
                    out=hp0[:, 0:B], in0=tr_sb[:, 0:B], scalar=1.0,
                    in1=tr_sb[:, B:2 * B], op0=OP.mult, op1=OP.add)
                nc.vector.scalar_tensor_tensor(
                    out=hp0[:, B:2 * B], in0=tr_sb[:, 2 * B:3 * B], scalar=1.0,
                    in1=tr_sb[:, 3 * B:4 * B], op0=OP.mult, op1=OP.add)
                part_sb = sb.tile([HW_W, B], f32, tag="part")
                nc.vector.scalar_tensor_tensor(
                    out=part_sb[:], in0=hp0[:, 0:B], scalar=1.0,
                    in1=hp0[:, B:2 * B], op0=OP.mult, op1=OP.add)
            else:
                part_sb = tr_sb
            nc.sync.dma_start(ar_in[:, :], part_sb[:, 0:B])

            # ============ AllGather partials + on-chip tree sum ============
            # (AllGather + 7 DVE adds beats AllReduce for a 9.7KB payload:
            # no reduce phase, ~half the collective latency)
            if collectives:
                nc.gpsimd.collective_compute(
                    "AllGather",
                    mybir.AluOpType.bypass,
                    replica_groups=[list(range(NC_CORES))],
                    ins=[ar_in[:].opt()],
                    outs=[ag_out[:].opt()],
                )
            else:
                for s in range(NC_CORES):
                    nc.sync.dma_start(ag_out[s, :, :], ar_in[:, :])
            red8_sb = sb.tile([HW_W, NC_CORES * B], f32, tag="red8")
            nc.sync.dma_start(
                red8_sb[:].rearrange("h (s b) -> h s b", s=NC_CORES, b=B),
                ag_out[:].rearrange("s h b -> h s b"),
            )
            for j in range(4):
                nc.vector.scalar_tensor_tensor(
                    out=red8_sb[:, j * B:(j + 1) * B],
                    in0=red8_sb[:, j * B:(j + 1) * B], scalar=1.0,
                    in1=red8_sb[:, (j + 4) * B:(j + 5) * B],
                    op0=OP.mult, op1=OP.add)
            for j in range(2):
                nc.vector.scalar_tensor_tensor(
                    out=red8_sb[:, j * B:(j + 1) * B],
                    in0=red8_sb[:, j * B:(j + 1) * B], scalar=1.0,
                    in1=red8_sb[:, (j + 2) * B:(j + 3) * B],
                    op0=OP.mult, op1=OP.add)
            red_sb = sb.tile([HW_W, B], f32, tag="red")
            nc.vector.scalar_tensor_tensor(
                out=red_sb[:], in0=red8_sb[:, 0:B], scalar=1.0,
                in1=red8_sb[:, B:2 * B], op0=OP.mult, op1=OP.add)

            # ================= final MLP + dueling combine =================
            adv_sb = sb.tile([12, B], f32, tag="adv")
            nc.scalar.activation(adv_sb[:], red_sb[64:76, :], AF.Relu, bias=advb_sb[:])
            # val path
            v1_sb = sb.tile([64, B], f32, tag="v1")
            nc.scalar.activation(v1_sb[:], red_sb[0:64, :], AF.Relu, bias=v1b_sb[:])
            psV = ps_t.tile([64, B], f32, tag="work")
            nc.tensor.matmul(psV[:], lhsT=v2w_sb[:], rhs=v1_sb[:], start=True, stop=True)
            v2_sb = sb.tile([64, B], f32, tag="v2")
            nc.scalar.activation(v2_sb[:], psV[:], AF.Relu, bias=v2b_sb[:])
            psV3 = ps_t.tile([1, B], f32, tag="work")
            nc.tensor.matmul(psV3[:], lhsT=v3w_sb[:], rhs=v2_sb[:], start=True, stop=True)
            val_sb = sb.tile([1, B], f32, tag="val")
            nc.vector.tensor_scalar_add(val_sb[:], psV3[:], v3b_sb[0:1, 0:1])
            # out = cmat.T @ adv + 1.T @ val
            psO = ps_t.tile([12, B], f32, tag="work")
            nc.tensor.matmul(psO[:], lhsT=cmat_sb[:], rhs=adv_sb[:], start=True, stop=False)
            nc.tensor.matmul(psO[:], lhsT=ones1[:, 0:12], rhs=val_sb[:], start=False, stop=True)
            out_sb = sb.tile([12, B], f32, tag="out")
            nc.vector.tensor_copy(out_sb[:], psO[:])
            nc.sync.dma_start(io["out"][:, :], out_sb[:])


# ---------------- driver ----------------

LAST_RESULTS = None

def _input_specs(shared, per_core):
    """name -> (shape, np dtype); per-core entries use per_core[0] shapes."""
    specs = {}
    for k, v in shared.items():
        specs[k] = v
    for k, v in per_core[0].items():
        specs[k] = v
    return specs


def kernel(**inputs) -> np.ndarray:
    import concourse.bacc as bacc
    import concourse.mybir as mybir
    import concourse.tile as tile
    from concourse import bass_utils

    shared, per_core, chunk_tile, nchunk = _prep_host(inputs)

    nc = bacc.Bacc("TRN2", target_bir_lowering=False, debug=False,
                   enable_asserts=False, num_devices=NC_CORES)

    io = {}
    specs = _input_specs(shared, per_core)
    for name, arr in specs.items():
        io[name] = nc.dram_tensor(
            name, list(arr.shape), mybir.dt.from_np(arr.dtype), kind="ExternalInput"
        ).ap()
    io["out"] = nc.dram_tensor(
        "out", [12, B], mybir.dt.float32, kind="ExternalOutput"
    ).ap()

    with tile.TileContext(nc) as tc:
        build_program(nc, tc, chunk_tile, nchunk, io)
    nc.compile()

    in_maps = []
    for j in range(NC_CORES):
        m = dict(shared)
        m.update(per_core[j])
        in_maps.append(m)

    res = bass_utils.run_bass_kernel_spmd(
        nc, in_maps, core_ids=list(range(NC_CORES)),
    )
    global LAST_RESULTS
    LAST_RESULTS = res
    out = res.results[0]["out"]                      # [12, 32]
    return out.T.reshape(B, 3, 4).copy().astype(np.float32)


if __name__ == "__main__":
    rng = np.random.default_rng(0)
    ei = rng.integers(0, N, (2, E)).astype(np.int64)
    demo = {
        "x": rng.standard_normal((B, N, F_IN), np.float32),
        "edge_index": ei,
        "edge_weight": rng.random(E, np.float32),
        "W1": rng.standard_normal((F_IN, F1), np.float32) / 4,
        "b1": np.zeros(F1, np.float32),
        "W2": rng.standard_normal((F1, F2), np.float32) / 11.3,
        "b2": np.zeros(F2, np.float32),
        "advW": rng.standard_normal((N * F2, 12), np.float32) / 1024,
        "advb": np.zeros(12, np.float32),
        "v1W": rng.standard_normal((N * F2, 64), np.float32) / 1024,
        "v1b": np.zeros(64, np.float32),
        "v2W": rng.standard_normal((64, 64), np.float32) / 8,
        "v2b": np.zeros(64, np.float32),
        "v3W": rng.standard_normal((64, 1), np.float32) / 8,
        "v3b": np.zeros(1, np.float32),
    }
    print(kernel(**demo).shape)



# revision 86
# speedup vs baseline: 1.5496x; 1.1287x over previous
"""Trainium2 Bass kernel for BHS_GCN: 2x GCNConv + dueling value/advantage heads.

Strategy (8 NeuronCores, single NEFF launch, bf16 compute / fp32 PSUM):
  - GCN phase batch-parallel: each core owns B_LOC=4 full graphs.
    Message passing = per-tile dma_gather of source-node rows (kept under the
    1024-descriptor SWDGE ring limit) + PE one-hot scatter-matmuls into PSUM
    (edges pre-sorted/packed by dst on host). Self-loop terms are NOT in the
    edge list: they are folded on DVE as agg += dinv2*H in the same op as the
    PSUM->SBUF copy (H1 kept in SBUF), cutting 20% of gather traffic.
  - mp1 gathers x as bf16 rows padded to 256B; layer matmuls and one-hot
    builds run at bf16 PE/DVE rates.
  - AllToAll (4 pipelined quarter-chunks, bf16) reshards the pre-W2
    aggregation to node-parallel: each core gets its 512-node slice for all
    32 batches, so each core streams only its 1/8 of advW/v1W (159MB bf16
    machine-wide, read once, unpadded 76-col tiles; first PRE_NB blocks
    prefetched into SBUF).
  - Head contraction is operand-swapped: the h2 slab [128k, 32sb] is the
    stationary operand (cheap weight loads) and headW streams; four k-slabs
    run concurrently in the PE's four 32-col groups (tile_position), with a
    single bank-wide dummy-clear then start=False accumulation (start=True
    clears has_written bank-wide).
  - AllGather of [76,32] partial head sums + on-chip tree-sum (cheaper than
    AllReduce for a 9.7KB payload); the tiny val-MLP and dueling combine run
    redundantly on every core; host takes core 0's output.
"""

import sys

sys.path.insert(0, "/opt/trn_rl_repo")

import os

import numpy as np
import ml_dtypes

# Precision mode: "f32" (exact), "bf16" (everything big in bf16), or a
# comma-set of {mp2,xfer,head}: mp2 = H1/messages/one-hots; xfer = A2A
# payload + W2; head = H2 + head weights. Accumulation is always fp32 PSUM.
PRECISION = os.environ.get("GCN_PREC", "bf16")
BF16 = np.dtype(ml_dtypes.bfloat16)


def _prec_groups():
    if PRECISION == "f32":
        return set()
    if PRECISION == "bf16":
        return {"mp2", "xfer", "head", "l1"}
    return set(PRECISION.split(","))


PREC_G = _prec_groups()

# ---------------- problem constants (hardcoded per contract) ----------------
B, N, F_IN, E = 32, 4096, 16, 16384
NC_CORES = 8
B_LOC = B // NC_CORES            # 4
NSLICE = N // NC_CORES           # 512 nodes per core for head phase
F1, F2 = 128, 256
P = 128
NTILES = N // P                  # 32 node tiles
BF1 = B_LOC * F_IN               # 64   (mp1 row width)
BFH = B_LOC * F1                 # 512  (H1 row width = mp2 gather width)
KTOT = NSLICE * F2               # 131072 contraction rows per core
KT = KTOT // P                   # 1024 K-tiles for head matmul
HW_W = 12 + 64                   # 76 head outputs (adv | v1)
# unpadded head-weight tiles: FWL would need 128 cols but the 68% extra
# HBM traffic costs more than the slower ldweights saves (DMA-bound kernel)
HW_P = HW_W
NT_HEAD = 16                     # nodes per W2/head block
PRE_NB = int(os.environ.get("GCN_PRE", "1"))   # head-weight nb-blocks in SBUF
# One dma_gather per node tile: a gather's descriptors must fit the 1024-slot
# SWDGE ring (single doorbell fires only after full emission — a gather
# bigger than the ring deadlocks on HW; the interpreter does not model this)
GSZ = int(os.environ.get("GCN_GSZ", "1"))
MSGBUF = int(os.environ.get("GCN_MSGBUF", "4"))  # msg/S pool depth
TILEPOS = os.environ.get("GCN_TILEPOS", "1") != "0"  # 4-way PE col tiling
N_CG = 4 if TILEPOS else 1       # PE col groups used by the head contraction
RHS_MERGE = os.environ.get("GCN_RHSMERGE", "1") != "0"  # single rhs-stage DMA
SRCSPLIT = os.environ.get("GCN_SRCSPLIT", "1") != "0"  # mp2 early/late gathers
# Interleaving A2A issue among the gathers REGRESSES on HW: the collective
# instruction occupies the GPSIMD queue for its whole transfer, stalling the
# gathers queued behind it. Keep collectives after all gathers (default off).
A2A_EARLY = os.environ.get("GCN_A2AEARLY", "0") != "0"
# Two A2A half-chunks (not four quarters): collective cost is dominated by a
# ~12-15us fixed overhead per call, so fewer, larger chunks win. mp2
# processes tiles half-major so chunk h's 16 tiles finish together.
N_A2A = 2
MP2_ORDER = [4 * k + 2 * h + w
             for h in range(N_A2A) for w in range(2) for k in range(8)]


def _pack_sorted(src_a, dst_a, nrm_a, tile_order=None, src_split=None):
    """Sort edges by dst, pack into 128-edge chunks such that every chunk's
    dsts fall in one 128-node tile. Tiles are packed in `tile_order` so that
    the chunks of GSZ consecutive tiles in that order are contiguous (one
    dma_gather per tile group). With src_split, each tile's chunks are packed
    early-src-first (src < src_split) so the early gather can read a
    DRAM slice that is ready before the whole H1 is written; returns the
    per-tile early-chunk count in that case."""
    order = np.argsort(dst_a, kind="stable")
    src_a, dst_a, nrm_a = src_a[order], dst_a[order], nrm_a[order]

    src_pk, nrm_pk, off_pk = [], [], []
    chunk_tile = []
    n_early = {}
    for t in (tile_order if tile_order is not None else range(NTILES)):
        sel = (dst_a >= t * P) & (dst_a < (t + 1) * P)
        s, d, w = src_a[sel], dst_a[sel], nrm_a[sel]
        if src_split is not None:
            parts = []
            for half, hsel in ((0, s < src_split), (1, s >= src_split)):
                sh, dh, wh = s[hsel], d[hsel], w[hsel]
                cnt = len(sh)
                nch_h = (cnt + P - 1) // P
                pad = nch_h * P - cnt
                parts.append((
                    np.concatenate([sh, np.zeros(pad, np.int64)]),
                    np.concatenate([wh, np.zeros(pad, np.float32)]),
                    np.concatenate([dh - t * P, np.zeros(pad, np.int64)]),
                    nch_h))
            if parts[0][3] + parts[1][3] == 0:
                parts[0] = (np.zeros(P, np.int64), np.zeros(P, np.float32),
                            np.zeros(P, np.int64), 1)
            n_early[t] = parts[0][3]
            src_pk.extend([parts[0][0], parts[1][0]])
            nrm_pk.extend([parts[0][1], parts[1][1]])
            off_pk.extend([parts[0][2], parts[1][2]])
            chunk_tile.extend([t] * (parts[0][3] + parts[1][3]))
            continue
        cnt = len(s)
        nch = max(1, (cnt + P - 1) // P)
        pad = nch * P - cnt
        src_pk.append(np.concatenate([s, np.zeros(pad, np.int64)]))
        nrm_pk.append(np.concatenate([w, np.zeros(pad, np.float32)]))
        off_pk.append(np.concatenate([d - t * P, np.zeros(pad, np.int64)]))
        chunk_tile.extend([t] * nch)

    src_pk = np.concatenate(src_pk)
    nrm_pk = np.concatenate(nrm_pk)
    off_pk = np.concatenate(off_pk)
    e_pad = len(src_pk)
    nchunk = e_pad // P
    assert nchunk == len(chunk_tile)

    # dma_gather index table: logical idx i lives at [i % 16, i // 16]
    gidx = np.zeros((P, e_pad // 16), np.int16)
    for p16 in range(16):
        gidx[p16, :] = src_pk[p16::16].astype(np.int16)
    gidx = np.tile(gidx[:16], (8, 1))  # replicate over all 128 partitions

    # per-chunk column tables: [p, c] = value of edge c*128+p
    nrm_t = nrm_pk.reshape(nchunk, P).T.copy()          # [128, nchunk] f32
    off_t = off_pk.reshape(nchunk, P).T.astype(np.float32).copy()
    return gidx, nrm_t, off_t, chunk_tile, nchunk, n_early


def _pack_edges(edge_index, edge_weight):
    """Two packings: mp1 includes self-loop edges (x stays in DRAM only);
    mp2 excludes them — the self-loop term is folded on DVE from the
    SBUF-resident H1 (saves 20% of gather traffic and chunk matmuls).
    Also returns dinv2 [128, NTILES]: 1/deg for node t*128+p."""
    src = np.asarray(edge_index[0], np.int64)
    dst = np.asarray(edge_index[1], np.int64)
    ew = np.asarray(edge_weight, np.float32)

    deg = np.zeros(N, np.float32)
    np.add.at(deg, dst, ew)
    deg += 1.0
    dinv = (1.0 / np.sqrt(deg)).astype(np.float32)
    norm = ew * dinv[src] * dinv[dst]

    # mp1: edges + self loops (src=dst=n, weight 1/deg[n])
    src_a = np.concatenate([src, np.arange(N, dtype=np.int64)])
    dst_a = np.concatenate([dst, np.arange(N, dtype=np.int64)])
    nrm_a = np.concatenate([norm, dinv * dinv]).astype(np.float32)
    t1 = _pack_sorted(src_a, dst_a, nrm_a)
    # mp2: edges only, packed in mp2's quarter-major processing order; each
    # tile's chunks are early-src-first so the early gather only depends on
    # the first half of H1
    t2 = _pack_sorted(src, dst, norm.astype(np.float32), tile_order=MP2_ORDER,
                      src_split=(N // 2 if SRCSPLIT else None))

    dinv2_t = (dinv * dinv).reshape(NTILES, P).T.copy()  # [128, NTILES]
    return t1, t2, dinv2_t


def _prep_host(inputs):
    """All host-side numpy preprocessing: edge packing, weight layout, batch shard."""
    x = np.asarray(inputs["x"], np.float32)
    (gidx, nrm_t, off_t, chunk_tile, nchunk, _), \
        (gidx2, nrm_t2, off_t2, chunk_tile2, nchunk2, n_early2), dinv2_t = \
        _pack_edges(inputs["edge_index"], inputs["edge_weight"])

    W1 = np.asarray(inputs["W1"], np.float32)      # [16,128]
    b1 = np.asarray(inputs["b1"], np.float32)      # [128]
    W2 = np.asarray(inputs["W2"], np.float32)      # [128,256]
    b2 = np.asarray(inputs["b2"], np.float32)      # [256]
    advW = np.asarray(inputs["advW"], np.float32)  # [N*256, 12]
    advb = np.asarray(inputs["advb"], np.float32)
    v1W = np.asarray(inputs["v1W"], np.float32)    # [N*256, 64]
    v1b = np.asarray(inputs["v1b"], np.float32)
    v2W = np.asarray(inputs["v2W"], np.float32)
    v2b = np.asarray(inputs["v2b"], np.float32)
    v3W = np.asarray(inputs["v3W"], np.float32)
    v3b = np.asarray(inputs["v3b"], np.float32)

    # W1 block-diagonal over the 4 local batches, plus a bias row driven by
    # a constant-1 row appended to aggT on device: [65, 512]
    w1bd = np.zeros((BF1 + 1, B_LOC * F1), np.float32)
    for b in range(B_LOC):
        w1bd[b * F_IN:(b + 1) * F_IN, b * F1:(b + 1) * F1] = W1
    w1bd[BF1, :] = np.tile(b1, B_LOC)

    # dueling combine matrix (adv part): out = C.T @ adv + val
    C = np.zeros((12, 12), np.float32)
    for h in range(3):
        for a in range(4):
            i = h * 4 + a
            C[i, i] += 1.0
            for a2 in range(4):
                C[h * 4 + a2, i] -= 0.25

    shared = {
        "gidx": gidx,
        "nrm_t": nrm_t.copy(),
        "off_t": off_t.copy(),
        "gidx2": gidx2,
        "nrm_t2": nrm_t2.copy(),
        "off_t2": off_t2.copy(),
        "dinv2_t": dinv2_t.copy(),
        "w1bd": (w1bd.astype(BF16) if "l1" in PREC_G else w1bd).copy(),
        "w2": (W2.astype(BF16) if "xfer" in PREC_G else W2).copy(),
        "b2c": b2[:, None].copy(),                  # [256,1]
        "advb_c": advb[:, None].copy(),             # [12,1]
        "v1b_c": v1b[:, None].copy(),               # [64,1]
        "v2w": v2W.copy(),                          # [64,64]
        "v2b_c": v2b[:, None].copy(),               # [64,1]
        "v3w": v3W.copy(),                          # [64,1]
        "v3b_c": v3b[None, :].copy(),               # [1,1]
        "cmat": C,
    }

    per_core = []
    for j in range(NC_CORES):
        # x batch-shard, node-major rows [N, b, f] -> [N, 64], bf16 padded to
        # 128 cols (gather elem_size_bytes must be a multiple of 256)
        x_nb = x[j * B_LOC:(j + 1) * B_LOC].transpose(1, 0, 2).reshape(N, BF1)
        x_loc = np.zeros((N, 2 * BF1), BF16)
        x_loc[:, :BF1] = x_nb.astype(BF16)
        # head weights: rows for this core's node slice, pre-tiled to
        # [128, KT*76]: col block j holds lhsT K-tile j = rows [128j,128j+128)
        r0 = j * KTOT
        aw = advW[r0:r0 + KTOT].reshape(KT, P, 12)
        vw = v1W[r0:r0 + KTOT].reshape(KT, P, 64)
        # v1 first (partitions 0:64), adv second (64:76): partition slices
        # must start at multiples of 32 on-device.
        hw = np.concatenate([vw, aw], axis=2)  # [KT, 128, 76]
        hw_t = hw.transpose(1, 0, 2).reshape(P, KT * HW_P)
        hw_t = (hw_t.astype(BF16) if "head" in PREC_G else hw_t).copy()
        per_core.append({"x_loc": x_loc, "headw_t": hw_t})

    return shared, per_core, (chunk_tile, chunk_tile2, n_early2), (nchunk, nchunk2)


# ---------------- device program ----------------

def build_program(nc, tc, chunk_tile, nchunk, io, collectives=True, phases=(1,1,1), repeat=1):
    """Emit the Tile program. io: dict of name -> DRAM AP."""
    import concourse.bass as bass
    import concourse.mybir as mybir
    import concourse.tile as tile
    from concourse.masks import make_identity

    f32 = mybir.dt.float32
    f32r = mybir.dt.float32r
    bf16 = mybir.dt.bfloat16
    mp2_dt = bf16 if "mp2" in PREC_G else f32
    xf_dt = bf16 if "xfer" in PREC_G else f32
    hd_dt = bf16 if "head" in PREC_G else f32
    l1_dt = bf16 if "l1" in PREC_G else f32
    i16 = mybir.dt.int16
    i32 = mybir.dt.int32
    AF = mybir.ActivationFunctionType
    OP = mybir.AluOpType

    chunk_tile1, chunk_tile2, n_early2 = chunk_tile
    nchunk1, nchunk2 = nchunk
    # chunks belonging to each node tile (contiguous ranges), per layer
    tile_chunks1 = [[] for _ in range(NTILES)]
    for c, t in enumerate(chunk_tile1):
        tile_chunks1[t].append(c)
    tile_chunks2 = [[] for _ in range(NTILES)]
    for c, t in enumerate(chunk_tile2):
        tile_chunks2[t].append(c)

    from contextlib import ExitStack
    with ExitStack() as ctx:
        const = ctx.enter_context(tc.tile_pool(name="const", bufs=1))
        sb = ctx.enter_context(tc.tile_pool(name="sb", bufs=3))
        sb_msg = ctx.enter_context(tc.tile_pool(name="msg", bufs=MSGBUF))
        sb_s = ctx.enter_context(tc.tile_pool(name="sbs", bufs=MSGBUF))
        sb_hw = ctx.enter_context(tc.tile_pool(name="sbhw", bufs=3))
        ps_agg = ctx.enter_context(tc.tile_pool(name="ps_agg", bufs=2, space="PSUM"))
        ps_t = ctx.enter_context(tc.tile_pool(name="ps_t", bufs=3, space="PSUM"))
        ps_tx = ctx.enter_context(tc.tile_pool(name="ps_tx", bufs=2, space="PSUM"))
        ps_head = ctx.enter_context(tc.tile_pool(name="ps_head", bufs=1, space="PSUM"))
        dram = ctx.enter_context(tc.tile_pool(name="dram", bufs=1, space="DRAM"))
        for _rep in range(repeat):
            # ---- constants into SBUF
            ident = const.tile([P, P], f32)
            make_identity(nc, ident[:])
            ident_x = const.tile([P, P], xf_dt)
            nc.vector.tensor_copy(ident_x[:], ident[:])
            iota_i = const.tile([P, P], i32)
            nc.gpsimd.iota(iota_i[:], pattern=[[1, P]], base=0, channel_multiplier=0)
            iota_f = const.tile([P, P], f32)
            nc.vector.tensor_copy(iota_f[:], iota_i[:])
            ones1 = const.tile([1, P], f32)
            nc.vector.memset(ones1[:], 1.0)

            gidx_sb = const.tile([P, nchunk1 * 8], i16)
            nc.sync.dma_start(gidx_sb[:], io["gidx"][:, :])
            nrm_sb = const.tile([P, nchunk1], f32)
            nc.sync.dma_start(nrm_sb[:], io["nrm_t"][:, :])
            off_sb = const.tile([P, nchunk1], f32)
            nc.sync.dma_start(off_sb[:], io["off_t"][:, :])
            gidx2_sb = const.tile([P, nchunk2 * 8], i16)
            nc.sync.dma_start(gidx2_sb[:], io["gidx2"][:, :])
            nrm2_sb = const.tile([P, nchunk2], f32)
            nc.sync.dma_start(nrm2_sb[:], io["nrm_t2"][:, :])
            off2_sb = const.tile([P, nchunk2], f32)
            nc.sync.dma_start(off2_sb[:], io["off_t2"][:, :])
            dinv2_sb = const.tile([P, NTILES], f32)
            nc.sync.dma_start(dinv2_sb[:], io["dinv2_t"][:, :])
            # SBUF-resident H1 (written by mp1, read by mp2's self-loop fold)
            h1_keep = const.tile([P, NTILES * BFH], mp2_dt)
            # head-weight prefetch: first PRE_NB nb-blocks' slabs (128*HW_P
            # cols each), loaded at kernel start to use DMA-idle time during
            # the PE-heavy GCN phase
            pre_cols = PRE_NB * 128 * HW_P
            hw_pre = const.tile([P, pre_cols], hd_dt)
            nc.sync.dma_start(hw_pre[:], io["headw_t"][:, 0:pre_cols])

            w1bd_sb = const.tile([BF1 + 1, B_LOC * F1], l1_dt)
            nc.sync.dma_start(w1bd_sb[:], io["w1bd"][:, :])
            w2_sb = const.tile([P, F2], xf_dt)
            nc.sync.dma_start(w2_sb[:], io["w2"][:, :])
            # b2 [256,1] -> two [128,1] sbuf column stacks
            b2a = const.tile([P, 2], f32)
            nc.sync.dma_start(b2a[:, 0:1], io["b2c"][0:P, :])
            nc.sync.dma_start(b2a[:, 1:2], io["b2c"][P:F2, :])
            advb_sb = const.tile([12, 1], f32)
            nc.sync.dma_start(advb_sb[:], io["advb_c"][:, :])
            v1b_sb = const.tile([64, 1], f32)
            nc.sync.dma_start(v1b_sb[:], io["v1b_c"][:, :])
            v2w_sb = const.tile([64, 64], f32)
            nc.sync.dma_start(v2w_sb[:], io["v2w"][:, :])
            v2b_sb = const.tile([64, 1], f32)
            nc.sync.dma_start(v2b_sb[:], io["v2b_c"][:, :])
            v3w_sb = const.tile([64, 1], f32)
            nc.sync.dma_start(v3w_sb[:], io["v3w"][:, :])
            v3b_sb = const.tile([1, 1], f32)
            nc.sync.dma_start(v3b_sb[:], io["v3b_c"][:, :])
            cmat_sb = const.tile([12, 12], f32)
            nc.sync.dma_start(cmat_sb[:], io["cmat"][:, :])

            # scratch DRAM
            h1_dram = dram.tile([N, BFH], mp2_dt)        # node-major H1
            # agg2 feature-major, one buffer pair per node-half so the
            # AllToAll pipelines with mp2 and the head phase:
            # a2a_*_q[h][k, fin, n256, b]
            a2a_in_q = [dram.tile([NC_CORES, F1, 2 * P, B_LOC], xf_dt,
                                  name=f"a2ain{h}") for h in range(N_A2A)]
            a2a_out_q = [dram.tile([NC_CORES, F1, 2 * P, B_LOC], xf_dt,
                                   name=f"a2aout{h}") for h in range(N_A2A)]
            ar_in = dram.tile([HW_W, B], f32)
            ag_out = dram.tile([NC_CORES, HW_W, B], f32)

            # ================= mp1 + L1 feature matmul =================
            def mp_layer(x_dram, elem, out_cb, tiles, gi_sb, nr_sb, of_sb,
                         dt=f32, order=None, elem_u=None, early_split=None,
                         hooks=None, swap_out=False):
                """gather + scatter for one GCN layer; out_cb(t, agg_psum_ap).

                One dma_gather covers GSZ consecutive tiles of the processing
                order (their chunks are packed contiguously). With
                early_split (maps tile -> early-chunk count), each tile's
                early chunks gather from the first half of x_dram only, so
                those gathers depend on half the producer writes and can
                hoist. hooks[i] is emitted after tile position i (used to
                interleave collective issue into this engine queue's program
                order). One-hot scatter matrices
                S[c][e, n] = norm[e] * (dstoff[e] == n) are built on DVE."""
                if elem_u is None:
                    elem_u = elem
                seq = list(order) if order is not None else list(range(NTILES))
                for gi in range(0, NTILES, GSZ):
                    gts = seq[gi:gi + GSZ]
                    g_cs = [c for t in gts for c in tiles[t]]
                    gc0, gnch = g_cs[0], len(g_cs)
                    assert g_cs == list(range(gc0, gc0 + gnch)), \
                        "group chunks must be contiguous (pack order mismatch)"
                    msg = sb_msg.tile([P, gnch * elem], dt, tag="msg")
                    msg3 = msg[:].rearrange("p (c e) -> p c e", e=elem)
                    if early_split is None:
                        nidx = gnch * P
                        nc.gpsimd.dma_gather(
                            out_ap=msg3,
                            in_ap=x_dram[:, :],
                            idxs_ap=gi_sb[:, gc0 * 8:(gc0 + gnch) * 8],
                            num_idxs=nidx,
                            num_idxs_reg=nidx,
                            elem_size=elem,
                        )
                    else:
                        assert GSZ == 1
                        t0 = gts[0]
                        ne = early_split[t0]
                        if ne > 0:
                            nc.gpsimd.dma_gather(
                                out_ap=msg3[:, 0:ne, :],
                                in_ap=x_dram[0:N // 2, :],
                                idxs_ap=gi_sb[:, gc0 * 8:(gc0 + ne) * 8],
                                num_idxs=ne * P,
                                num_idxs_reg=ne * P,
                                elem_size=elem,
                            )
                        if ne < gnch:
                            nc.gpsimd.dma_gather(
                                out_ap=msg3[:, ne:gnch, :],
                                in_ap=x_dram[:, :],
                                idxs_ap=gi_sb[:, (gc0 + ne) * 8:(gc0 + gnch) * 8],
                                num_idxs=(gnch - ne) * P,
                                num_idxs_reg=(gnch - ne) * P,
                                elem_size=elem,
                            )
                    for t in gts:
                        cs = tiles[t]
                        nch = len(cs)
                        s_t = sb_s.tile([P, nch * P], dt, tag="sC")
                        for i, c in enumerate(cs):
                            # S[e, n] = (iota[n] == dstoff[e]) * norm[e]
                            nc.vector.tensor_scalar(
                                out=s_t[:, i * P:(i + 1) * P], in0=iota_f[:],
                                scalar1=of_sb[:, c:c + 1], scalar2=nr_sb[:, c:c + 1],
                                op0=OP.is_equal, op1=OP.mult,
                            )
                        if swap_out:
                            # transposed aggregate [elem_u, 128n] directly:
                            # lhsT = msg chunk (stationary), rhs = S (moving)
                            agg = ps_agg.tile([elem_u, P], f32, tag="agg")
                            for i, c in enumerate(cs):
                                mo = c - gc0
                                nc.tensor.matmul(
                                    agg[:],
                                    lhsT=msg[:, mo * elem:mo * elem + elem_u],
                                    rhs=s_t[:, i * P:(i + 1) * P],
                                    start=(i == 0),
                                    stop=(i == nch - 1),
                                )
                        else:
                            agg = ps_agg.tile([P, elem_u], f32, tag="agg")
                            for i, c in enumerate(cs):
                                mo = c - gc0
                                nc.tensor.matmul(
                                    agg[:],
                                    lhsT=s_t[:, i * P:(i + 1) * P],
                                    rhs=msg[:, mo * elem:mo * elem + elem_u],
                                    start=(i == 0),
                                    stop=(i == nch - 1),
                                )
                        out_cb(t, agg)
                    if hooks is not None:
                        for pos in range(gi, gi + GSZ):
                            if pos in hooks:
                                hooks[pos]()

            # ---- layer 1 (scatter matmul emits aggT [64, 128n] directly —
            # no PE transpose needed; last aggT row carries a constant 1 that
            # drives the b1 bias row of w1bd)
            def l1_out(t, aggT_ps):
                aggT = sb.tile([BF1 + 1, P], l1_dt, tag="aggT1")
                nc.vector.tensor_copy(aggT[0:BF1, :], aggT_ps[:])
                nc.vector.memset(aggT[BF1:BF1 + 1, :], 1.0)
                # H1[t] = relu(aggT_aug.T @ w1bd_aug)
                psH = ps_t.tile([P, B_LOC * F1], f32, tag="work")
                nc.tensor.matmul(psH[:], lhsT=aggT[:], rhs=w1bd_sb[:], start=True, stop=True)
                h1slice = h1_keep[:, t * BFH:(t + 1) * BFH]
                nc.scalar.activation(h1slice, psH[:], AF.Relu)
                nc.sync.dma_start(h1_dram[t * P:(t + 1) * P, :], h1slice)

            if phases[0]:
                # x rows padded to 128 bf16 cols (256B gather min); matmuls
                # only consume the first 64
                mp_layer(io["x_loc"], 2 * BF1, l1_out, tile_chunks1,
                         gidx_sb, nrm_sb, off_sb, dt=bf16, elem_u=BF1,
                         swap_out=True)

            # ---- layer 2 message passing -> a2a_in (feature-major [k, fin, n', b]
            # so the post-A2A W2-rhs reads are contiguous per partition)
            def l2_out(t, agg):
                # agg_sb = agg + dinv2[t] * H1[t]  (self-loop fold, one DVE
                # op); output already in the A2A's bf16 — the payload is bf16
                # anyway, and bf16 PE transposes run 2x faster than f32
                agg_sb = sb.tile([P, BFH], xf_dt, tag="agg2sb")
                nc.vector.scalar_tensor_tensor(
                    out=agg_sb[:], in0=h1_keep[:, t * BFH:(t + 1) * BFH],
                    scalar=dinv2_sb[:, t:t + 1], in1=agg[:],
                    op0=OP.mult, op1=OP.add,
                )
                a2a_sb = sb.tile([P, BFH], xf_dt, tag="a2asb")
                a2a_3d = a2a_sb[:].rearrange("f (n b) -> f n b", b=B_LOC)
                for b in range(B_LOC):
                    psT = ps_tx.tile([P, P], xf_dt, tag="workx")
                    nc.tensor.transpose(psT[:], agg_sb[:, b * F1:(b + 1) * F1], ident_x[:])
                    nc.vector.tensor_copy(a2a_3d[:, :, b], psT[:])
                k, r = t // 4, t % 4
                h, w = r // 2, r % 2
                nc.sync.dma_start(a2a_in_q[h][k, :, w * P:(w + 1) * P, :],
                                  a2a_sb[:])

            def a2a_chunk(q):
                if collectives:
                    nc.gpsimd.collective_compute(
                        "AllToAll",
                        mybir.AluOpType.bypass,
                        replica_groups=[list(range(NC_CORES))],
                        ins=[a2a_in_q[q][:].opt()],
                        outs=[a2a_out_q[q][:].opt()],
                    )
                else:
                    for s in range(NC_CORES):
                        nc.sync.dma_start(
                            a2a_out_q[q][s].rearrange("f n b -> f (n b)"),
                            a2a_in_q[q][s].rearrange("f n b -> f (n b)"),
                        )

            if phases[1]:
                # quarter-major order: all of quarter q's tiles finish
                # together, releasing A2A chunk h while mp2 continues on
                # half h+1 (A2A_EARLY interleave kept available but off —
                # collectives block the Pool queue for their full transfer).
                hooks = None
                if A2A_EARLY:
                    hooks = {20: lambda: a2a_chunk(0),
                             31: lambda: a2a_chunk(1)}
                mp_layer(h1_dram, BFH, l2_out, tile_chunks2,
                         gidx2_sb, nrm2_sb, off2_sb, dt=mp2_dt,
                         order=MP2_ORDER,
                         early_split=(n_early2 if SRCSPLIT else None),
                         hooks=hooks)

            # ====== per-quarter AllToAll chunk + W2 + head partials ======
            # Head contraction runs operand-swapped: the h2 slab [128k, 32sb]
            # is the stationary operand (cheap 32-col weight loads) and the
            # headW slab [128k, 76] streams. Four consecutive k-slabs occupy
            # the PE's four 32-wide column groups (tile_position) and run
            # concurrently; ps_hd[32j:32j+32, :] holds col-group j's partial.
            NB_H = 64
            SB_COLS = NB_H * B_LOC                       # 256 cols per src core
            ps_hd = ps_head.tile([P, HW_W], f32)
            nblocks = (NSLICE // NB_H) if phases[2] else 0
            n_slab = nblocks * NB_H * 2
            slab_ctr = 0
            if not phases[2]:
                nc.vector.memset(ps_hd[:], 0.0)
            elif TILEPOS:
                # col-group accumulation shares one bank; start=True clears
                # has_written BANK-wide, so zero the whole region once with a
                # dummy matmul and accumulate everything else with start=False
                zero_w = const.tile([P, P], f32)
                nc.vector.memset(zero_w[:], 0.0)
                nc.tensor.matmul(ps_hd[:], lhsT=zero_w[:],
                                 rhs=ident[:, 0:HW_W], start=True, stop=False,
                                 skip_group_check=True)

            for nb in range(nblocks):
                h, seg = nb // 4, nb % 4
                if seg == 0 and not (A2A_EARLY and phases[1]):
                    a2a_chunk(h)
                # stage rhs [128 fin, (s, n, b)]: 512B runs per partition
                rhs_sb = sb.tile([P, NC_CORES * SB_COLS], xf_dt, tag="w2rhs")
                if RHS_MERGE:
                    nc.sync.dma_start(
                        rhs_sb[:].rearrange("p (s n b) -> p s n b",
                                            s=NC_CORES, n=NB_H, b=B_LOC),
                        a2a_out_q[h][:, :, seg * NB_H:(seg + 1) * NB_H, :]
                        .rearrange("s f n b -> f s n b"),
                    )
                else:
                    for s in range(NC_CORES):
                        nc.sync.dma_start(
                            rhs_sb[:, s * SB_COLS:(s + 1) * SB_COLS],
                            a2a_out_q[h][s, :, seg * NB_H:(seg + 1) * NB_H, :],
                        )
                h2 = []
                for fh in range(2):
                    # h2 stored node-major (n, s, b) so the head lhsT slab
                    # [128, 32] for node i is a contiguous column slice
                    h2sb = sb.tile([P, NC_CORES * SB_COLS], hd_dt, tag="h2sb")
                    h2v = h2sb[:].rearrange("p (n s b) -> p n s b",
                                            n=NB_H, s=NC_CORES, b=B_LOC)
                    for qq in range(4):  # free split: 512-col matmuls
                        sl = slice(qq * 512, (qq + 1) * 512)
                        psW = ps_t.tile([P, 512], f32, tag="work")
                        nc.tensor.matmul(
                            psW[:], lhsT=w2_sb[:, fh * P:(fh + 1) * P],
                            rhs=rhs_sb[:, sl], start=True, stop=True,
                        )
                        # relu + per-partition bias b2[fh*128 + p]; psW's
                        # column order is (s, n, b) for s in {2qq, 2qq+1}
                        nc.scalar.activation(
                            h2v[:, :, 2 * qq:2 * qq + 2, :]
                            .rearrange("p n s b -> p s n b"),
                            psW[:].rearrange("p (s n b) -> p s n b",
                                             s=2, n=NB_H, b=B_LOC),
                            AF.Relu, bias=b2a[:, fh:fh + 1])
                    h2.append(h2sb)
                hw_sb = None
                for i in range(NB_H):
                    g = (nb * NB_H + i) // 16          # 16-node headW slab index
                    if nb < PRE_NB:
                        hw_view = hw_pre[:, g * 32 * HW_P:(g + 1) * 32 * HW_P]
                    else:
                        if i % 16 == 0:
                            hw_sb = sb_hw.tile([P, 32 * HW_P], hd_dt, tag="hwslab")
                            nc.sync.dma_start(
                                hw_sb[:],
                                io["headw_t"][:, g * 32 * HW_P:(g + 1) * 32 * HW_P])
                        hw_view = hw_sb[:]
                    for fh in range(2):
                        jj = 2 * (i % 16) + fh
                        j4 = slab_ctr % N_CG
                        nc.tensor.matmul(
                            ps_hd[32 * j4:32 * (j4 + 1), :],
                            lhsT=h2[fh][:, i * 32:(i + 1) * 32],
                            rhs=hw_view[:, jj * HW_P:(jj + 1) * HW_P],
                            start=(not TILEPOS) and slab_ctr == 0,
                            stop=(slab_ctr >= n_slab - N_CG),
                            tile_position=(0, 32 * j4) if TILEPOS else None,
                            skip_group_check=True,
                        )
                        slab_ctr += 1

            # ps_hd [N_CG*32b, 76] -> transpose -> [76, N_CG*32b] -> sum groups
            hd_sb = sb.tile([32 * N_CG, HW_W], f32, tag="hdsb")
            nc.vector.tensor_copy(hd_sb[:], ps_hd[0:32 * N_CG, :])
            psTr = ps_t.tile([HW_W, 32 * N_CG], f32, tag="work")
            nc.tensor.transpose(psTr[:], hd_sb[:],
                                ident[0:32 * N_CG, 0:32 * N_CG])
            tr_sb = sb.tile([HW_W, 32 * N_CG], f32, tag="trsb")
            nc.vector.tensor_copy(tr_sb[:], psTr[:])
            if N_CG == 4:
                hp0 = sb.tile([HW_W, 2 * B], f32, tag="hp0")
                nc.vector.scalar_tensor_tensor(
                    out=hp0[:, 0:B], in0=tr_sb[:, 0:B], scalar=1.0,
                    in1=tr_sb[:, B:2 * B], op0=OP.mult, op1=OP.add)
                nc.vector.scalar_tensor_tensor(
                    out=hp0[:, B:2 * B], in0=tr_sb[:, 2 * B:3 * B], scalar=1.0,
                    in1=tr_sb[:, 3 * B:4 * B], op0=OP.mult, op1=OP.add)
                part_sb = sb.tile([HW_W, B], f32, tag="part")
                nc.vector.scalar_tensor_tensor(
                    out=part_sb[:], in0=hp0[:, 0:B], scalar=1.0,
                    in1=hp0[:, B:2 * B], op0=OP.mult, op1=OP.add)
            else:
                part_sb = tr_sb
            nc.sync.dma_start(ar_in[:, :], part_sb[:, 0:B])

            # ============ AllGather partials + on-chip tree sum ============
            # (AllGather + 7 DVE adds beats AllReduce for a 9.7KB payload:
            # no reduce phase, ~half the collective latency)
            if collectives:
                nc.gpsimd.collective_compute(
                    "AllGather",
                    mybir.AluOpType.bypass,
                    replica_groups=[list(range(NC_CORES))],
                    ins=[ar_in[:].opt()],
                    outs=[ag_out[:].opt()],
                )
            else:
                for s in range(NC_CORES):
                    nc.sync.dma_start(ag_out[s, :, :], ar_in[:, :])
            red8_sb = sb.tile([HW_W, NC_CORES * B], f32, tag="red8")
            nc.sync.dma_start(
                red8_sb[:].rearrange("h (s b) -> h s b", s=NC_CORES, b=B),
                ag_out[:].rearrange("s h b -> h s b"),
            )
            for j in range(4):
                nc.vector.scalar_tensor_tensor(
                    out=red8_sb[:, j * B:(j + 1) * B],
                    in0=red8_sb[:, j * B:(j + 1) * B], scalar=1.0,
                    in1=red8_sb[:, (j + 4) * B:(j + 5) * B],
                    op0=OP.mult, op1=OP.add)
            for j in range(2):
                nc.vector.scalar_tensor_tensor(
                    out=red8_sb[:, j * B:(j + 1) * B],
                    in0=red8_sb[:, j * B:(j + 1) * B], scalar=1.0,
                    in1=red8_sb[:, (j + 2) * B:(j + 3) * B],
                    op0=OP.mult, op1=OP.add)
            red_sb = sb.tile([HW_W, B], f32, tag="red")
            nc.vector.scalar_tensor_tensor(
                out=red_sb[:], in0=red8_sb[:, 0:B], scalar=1.0,
                in1=red8_sb[:, B:2 * B], op0=OP.mult, op1=OP.add)

            # ================= final MLP + dueling combine =================
            adv_sb = sb.tile([12, B], f32, tag="adv")
            nc.scalar.activation(adv_sb[:], red_sb[64:76, :], AF.Relu, bias=advb_sb[:])
            # val path
            v1_sb = sb.tile([64, B], f32, tag="v1")
            nc.scalar.activation(v1_sb[:], red_sb[0:64, :], AF.Relu, bias=v1b_sb[:])
            psV = ps_t.tile([64, B], f32, tag="work")
            nc.tensor.matmul(psV[:], lhsT=v2w_sb[:], rhs=v1_sb[:], start=True, stop=True)
            v2_sb = sb.tile([64, B], f32, tag="v2")
            nc.scalar.activation(v2_sb[:], psV[:], AF.Relu, bias=v2b_sb[:])
            psV3 = ps_t.tile([1, B], f32, tag="work")
            nc.tensor.matmul(psV3[:], lhsT=v3w_sb[:], rhs=v2_sb[:], start=True, stop=True)
            val_sb = sb.tile([1, B], f32, tag="val")
            nc.vector.tensor_scalar_add(val_sb[:], psV3[:], v3b_sb[0:1, 0:1])
            # out = cmat.T @ adv + 1.T @ val
            psO = ps_t.tile([12, B], f32, tag="work")
            nc.tensor.matmul(psO[:], lhsT=cmat_sb[:], rhs=adv_sb[:], start=True, stop=False)
            nc.tensor.matmul(psO[:], lhsT=ones1[:, 0:12], rhs=val_sb[:], start=False, stop=True)
            out_sb = sb.tile([12, B], f32, tag="out")
            nc.vector.tensor_copy(out_sb[:], psO[:])
            nc.sync.dma_start(io["out"][:, :], out_sb[:])


# ---------------- driver ----------------

LAST_RESULTS = None

def _input_specs(shared, per_core):
    """name -> (shape, np dtype); per-core entries use per_core[0] shapes."""
    specs = {}
    for k, v in shared.items():
        specs[k] = v
    for k, v in per_core[0].items():
        specs[k] = v
    return specs


def kernel(**inputs) -> np.ndarray:
    import concourse.bacc as bacc
    import concourse.mybir as mybir
    import concourse.tile as tile
    from concourse import bass_utils

    shared, per_core, chunk_tile, nchunk = _prep_host(inputs)

    nc = bacc.Bacc("TRN2", target_bir_lowering=False, debug=False,
                   enable_asserts=False, num_devices=NC_CORES)

    io = {}
    specs = _input_specs(shared, per_core)
    for name, arr in specs.items():
        io[name] = nc.dram_tensor(
            name, list(arr.shape), mybir.dt.from_np(arr.dtype), kind="ExternalInput"
        ).ap()
    io["out"] = nc.dram_tensor(
        "out", [12, B], mybir.dt.float32, kind="ExternalOutput"
    ).ap()

    with tile.TileContext(nc) as tc:
        build_program(nc, tc, chunk_tile, nchunk, io)
    nc.compile()

    in_maps = []
    for j in range(NC_CORES):
        m = dict(shared)
        m.update(per_core[j])
        in_maps.append(m)

    res = bass_utils.run_bass_kernel_spmd(
        nc, in_maps, core_ids=list(range(NC_CORES)),
    )
    global LAST_RESULTS
    LAST_RESULTS = res
    out = res.results[0]["out"]                      # [12, 32]
    return out.T.reshape(B, 3, 4).copy().astype(np.float32)


if __name__ == "__main__":
    rng = np.random.default_rng(0)
    ei = rng.integers(0, N, (2, E)).astype(np.int64)
    demo = {
        "x": rng.standard_normal((B, N, F_IN), np.float32),
        "edge_index": ei,
        "edge_weight": rng.random(E, np.float32),
        "W1": rng.standard_normal((F_IN, F1), np.float32) / 4,
        "b1": np.zeros(F1, np.float32),
        "W2": rng.standard_normal((F1, F2), np.float32) / 11.3,
        "b2": np.zeros(F2, np.float32),
        "advW": rng.standard_normal((N * F2, 12), np.float32) / 1024,
        "advb": np.zeros(12, np.float32),
        "v1W": rng.standard_normal((N * F2, 64), np.float32) / 1024,
        "v1b": np.zeros(64, np.float32),
        "v2W": rng.standard_normal((64, 64), np.float32) / 8,
        "v2b": np.zeros(64, np.float32),
        "v3W": rng.standard_normal((64, 1), np.float32) / 8,
        "v3b": np.zeros(1, np.float32),
    }
    print(kernel(**demo).shape)

